# revision 1
# baseline (speedup 1.0000x reference)
"""Trainium2 Bass kernel: per-head attention + residual + LayerNorm.

Problem shape: x [4, 2048, 1024], 16 heads of dk=64, causal softmax attention
with per-head Q/K/V linear projections, residual add, LayerNorm(D).

Sharding (8 cores): head-parallel. Core i owns heads (2i, 2i+1), i.e. feature
columns 128*i : 128*(i+1). Each core computes its feature slice of the output;
the only cross-core communication is a tiny per-batch AllReduce of LayerNorm
partial sums (sum and sum-of-squares over each core's 128 features). The host
shards/gathers and pre-transposes x (the [dk, S] layout each head needs).

Per-core kernel design (bf16 matmuls, fp32 PSUM accumulation):
- Scores via a host-precomputed bilinear form: scores^T = xh_aug^T @ z with
  z = [M @ xh^T + u | beta-row], M = Wk (Wq/sqrt(dk))^T. One projected tensor
  (z) instead of Q and K halves the PSUM->SBUF copies; x^T itself (with a
  built-in ones row for the bias terms) is the stationary matmul operand.
- Flash-style t-outer loop over 1024-col query halves; scores accumulate in
  PSUM [128,1024] chunks on a dedicated 3-deep ring (6 banks) that serves
  ONLY the score->exp pipeline; exp runs on ScalarE straight from PSUM in
  one instruction per (t-block, half), P is bf16 in SBUF. All other PSUM
  users (split O^T accumulators, projection staging, epilogue transposes)
  live on a separate ring of fast-release 1-bank [*,512] slots (2 banks),
  so no phase ever blocks the score pipeline's buffers.
- Causal mask: an identity-matmul accumulates -40 onto the upper triangle of
  the diagonal 128-block before exp (no vector-engine masking); sub-diagonal
  dead zones are simply never read by PV.
- PV accumulates O^T [65,1024] in PSUM with a ones-augmented V, so softmax
  denominators ride along as row 64; per 128-tile PE-transposes then let a
  single fused DVE op do (O*1/l + x) with the row-sum accumulated for free.
- rstd = exp(-0.5*ln(var+eps)) keeps every activation in one ACT table set
  (natural_log_exp_and_others; enforced by filtering the set map at compile).
- Emission is software-pipelined: the next pair's z/V projection is emitted
  mid-way through the current score loop, the transpose/normalize epilogue is
  deferred one unit, and per-half LayerNorm stats AllReduce in 8 small chunks
  so TensorE/ScalarE/VectorE and the collective overlap across units.

Self-contained: hardcodes all shapes; no sibling imports.
"""

import os
import numpy as np
import ml_dtypes

import concourse.bass as bass
import concourse.bacc as bacc
import concourse.mybir as mybir
from concourse.tile import TileContext
from concourse.bass_utils import run_bass_kernel_spmd

B, S, D, H = 4, 2048, 1024, 16
NCORES = 8
HPC = H // NCORES          # heads per core = 2
DK = D // H                # 64
DC = HPC * DK              # 128 feature cols per core
NT = S // 128              # 16 row tiles of 128
EPS = 1e-5
MASKNEG = -40.0
SPBUFS = 3
OPBUFS = 1
EPI_LAG = 1
HOOKJ = 6
QKBUFS = 3
PBUFS = 6
BF = mybir.dt.bfloat16
F32 = mybir.dt.float32
BF_NP = ml_dtypes.bfloat16
RG = [list(range(NCORES))]
A = mybir.AluOpType
AF = mybir.ActivationFunctionType

LAST_RESULTS = None  # BassKernelResults of the last run (for test harness)


def _build_graph(apply_affine: bool, B_: int = B, S_: int = S, rg=None, fake_ar: bool = False) -> bass.Bass:
    nc = bacc.Bacc()
    NT_ = S_ // 128
    if rg is None:
        rg = RG

    xt = nc.declare_dram_parameter("xt", [B_, HPC, DK + 1, S_], BF, isOutput=False)
    xs = nc.declare_dram_parameter("xs", [B_, S_, DC], F32, isOutput=False)
    wpack = nc.declare_dram_parameter(
        "wpack", [DK, HPC * (DK + 1) + HPC * DK], BF, isOutput=False
    )
    zb = nc.declare_dram_parameter("zb", [DK + 1, HPC], F32, isOutput=False)
    bv16 = nc.declare_dram_parameter("bv16", [HPC, 128, 16 * DK], F32, isOutput=False)
    if apply_affine:
        gam = nc.declare_dram_parameter("gam", [128, DC], F32, isOutput=False)
        bet = nc.declare_dram_parameter("bet", [128, DC], F32, isOutput=False)
    out = nc.declare_dram_parameter("out", [B_, S_, DC], F32, isOutput=True)

    # constants baked into the NEFF
    idn_h = nc.inline_tensor(np.eye(DK + 1, dtype=np.float32), name="idn")
    trineg_np = np.where(
        np.arange(128)[:, None] > np.arange(128)[None, :], MASKNEG, 0.0
    ).astype(np.float32)
    imask_h = nc.inline_tensor(
        np.concatenate([np.eye(128, dtype=np.float32), trineg_np], axis=1).astype(
            BF_NP
        ),
        name="imask",
    )

    # collective bounce buffers: LayerNorm stats per (batch, s-half):
    # [2(sum,sumsq), 128 rows, tiles-in-half]
    NHALF = (S_ + 1023) // 1024
    NTH = NT_ // NHALF
    stats_in = nc.dram_tensor("stats_in", [B_, NHALF, 2, 128, NTH], F32)
    stats_out = nc.dram_tensor(
        "stats_out", [B_, NHALF, 2, 128, NTH], F32, addr_space="Shared"
    )

    with TileContext(nc) as tc:
        with (
            tc.tile_pool(name="consts", bufs=1) as cpool,
            tc.tile_pool(name="sb", bufs=2) as sb,
            tc.tile_pool(name="ps", bufs=1, space="PSUM") as ps,
        ):
            # ---- load constants (single coalesced DMAs, first-needed first) ----
            wp_t = cpool.tile([DK, HPC * (DK + 1) + HPC * DK], BF, tag="wp")
            nc.sync.dma_start(out=wp_t[:], in_=wpack[:, :])
            zw_t = wp_t[:][:, 0 : HPC * (DK + 1)]
            wv_t = wp_t[:][:, HPC * (DK + 1) : HPC * (DK + 1) + HPC * DK]
            # queue order tuned for the first exp: wpack, then the first
            # half of head-0 x^T (all the first projection needs), then the
            # small constants the first z-copy and diagonal mask need, then
            # the rest of batch-0 x^T
            xth0 = [
                sb.tile([DK + 1, S_], BF, tag="xth", name=f"xth0_{h2}", bufs=B_ * HPC)
                for h2 in range(HPC)
            ]
            nc.sync.dma_start(out=xth0[0][:, 0 : S_ // 2], in_=xt[0, 0, :, 0 : S_ // 2])
            zbq_t = cpool.tile([DK + 1, HPC], F32, tag="zb")
            nc.sync.dma_start(out=zbq_t[:], in_=zb[:, :])
            imaskq_t = cpool.tile([128, 256], BF, tag="imask")
            nc.sync.dma_start(out=imaskq_t[:], in_=imask_h[:, :])
            nc.sync.dma_start(out=xth0[0][:, S_ // 2 : S_], in_=xt[0, 0, :, S_ // 2 : S_])
            nc.sync.dma_start(out=xth0[1][:, 0 : S_ // 2], in_=xt[0, 1, :, 0 : S_ // 2])
            nc.sync.dma_start(out=xth0[1][:, S_ // 2 : S_], in_=xt[0, 1, :, S_ // 2 : S_])
            idn_t = cpool.tile([DK + 1, DK + 1], F32, tag="idn")
            nc.gpsimd.dma_start(out=idn_t[:], in_=idn_h[:, :])
            bv16_t = cpool.tile([128, HPC * 16 * DK], F32, tag="bv16")
            for h in range(HPC):
                nc.gpsimd.dma_start(
                    out=bv16_t[:, 16 * DK * h : 16 * DK * (h + 1)], in_=bv16[h]
                )
            if apply_affine:
                gam_t = cpool.tile([128, DC], F32, tag="gam")
                nc.sync.dma_start(out=gam_t[:], in_=gam[:, :])
                bet_t = cpool.tile([128, DC], F32, tag="bet")
                nc.sync.dma_start(out=bet_t[:], in_=bet[:, :])

            zb_t = zbq_t
            idn128_t = imaskq_t[:][:, 0:128]
            maskt_t = imaskq_t[:][:, 128:256]
            eps_t = cpool.tile([128, 1], F32, tag="eps")
            nc.vector.memset(eps_t[:], EPS)

            pending_epi = [None]

            def _emit_stats(b, y_b, acc, hs, he, ch):
                # LayerNorm partial stats + AllReduce for one s-half
                t0, t1 = hs // 128, he // 128
                nth = t1 - t0
                sums = sb.tile([128, NTH], F32, tag="sums", bufs=3)
                nc.vector.tensor_add(
                    sums[:, 0:nth], acc[0][:, t0:t1], acc[1][:, t0:t1]
                )
                sq = sb.tile([128, NTH], F32, tag="sq", bufs=3)
                for i in range(t0, t1):
                    scr = sb.tile([128, 128], F32, tag="scr")
                    nc.vector.scalar_tensor_tensor(
                        scr[:],
                        y_b[:, 128 * i : 128 * i + 128],
                        1.0,
                        y_b[:, 128 * i : 128 * i + 128],
                        A.mult,
                        A.mult,
                        accum_out=sq[:, i - t0 : i - t0 + 1],
                    )
                nc.sync.dma_start(out=stats_in[b, ch, 0], in_=sums[:, 0:nth])
                nc.sync.dma_start(out=stats_in[b, ch, 1], in_=sq[:, 0:nth])
                if fake_ar:
                    nc.sync.dma_start(out=stats_out[b, ch], in_=stats_in[b, ch])
                else:
                    nc.gpsimd.collective_compute(
                        "AllReduce",
                        A.add,
                        replica_groups=rg,
                        ins=[stats_in[b, ch].opt()],
                        outs=[stats_out[b, ch].opt()],
                    )

            def emit_ln(b, ch, y_b):
                t0 = ch * NTH
                red = sb.tile([128, 2 * NTH], F32, tag="red", bufs=3)
                nc.sync.dma_start(
                    out=red[:].rearrange("p (c t) -> p c t", t=NTH),
                    in_=stats_out[b, ch].rearrange("c p t -> p c t"),
                )
                mean = sb.tile([128, NTH], F32, tag="mean", bufs=3)
                nc.vector.tensor_scalar(
                    mean[:], red[:, 0:NTH], 1.0 / D, None, A.mult
                )
                msq = sb.tile([128, NTH], F32, tag="msq", bufs=3)
                nc.vector.tensor_mul(msq[:], mean[:], mean[:])
                var = sb.tile([128, NTH], F32, tag="var", bufs=3)
                nc.vector.scalar_tensor_tensor(
                    var[:], red[:, NTH : 2 * NTH], 1.0 / D, msq[:], A.mult,
                    A.subtract,
                )
                lnv = sb.tile([128, NTH], F32, tag="lnv", bufs=3)
                nc.scalar.activation(lnv[:], var[:], AF.Ln, bias=eps_t[:])
                rstd = sb.tile([128, NTH], F32, tag="rstd", bufs=3)
                nc.scalar.activation(rstd[:], lnv[:], AF.Exp, scale=-0.5)
                ostb = sb.tile([128, 128 * NTH], F32, tag="ost", bufs=2)
                for k in range(NTH):
                    i = t0 + k
                    nc.vector.tensor_scalar(
                        ostb[:, 128 * k : 128 * k + 128],
                        y_b[:, 128 * i : 128 * i + 128],
                        mean[:, k : k + 1],
                        rstd[:, k : k + 1],
                        A.subtract,
                        A.mult,
                    )
                    if apply_affine:
                        nc.vector.tensor_mul(
                            ostb[:, 128 * k : 128 * k + 128],
                            ostb[:, 128 * k : 128 * k + 128],
                            gam_t[:],
                        )
                        nc.vector.tensor_add(
                            ostb[:, 128 * k : 128 * k + 128],
                            ostb[:, 128 * k : 128 * k + 128],
                            bet_t[:],
                        )
                eng = nc.gpsimd if ((b * NHALF + ch) % 2 == 0 and b < B_ - 1) else nc.sync
                eng.dma_start(
                    out=out[b, 128 * t0 : 128 * (t0 + NTH), :].rearrange(
                        "(i p) d -> p i d", p=128
                    ),
                    in_=ostb[:].rearrange("p (i d) -> p i d", d=128),
                )

            y_tiles = {}
            bstate = {}
            pstate = {}
            pw = min(1024, S_)
            NP = B_ * HPC

            def emit_proj(pair):
                b, hh = divmod(pair, HPC)
                if hh == 0:
                    if b == 0:
                        xth = xth0
                    else:
                        xth = [None, None]
                        for h2 in range(HPC):
                            xth[h2] = sb.tile(
                                [DK + 1, S_], BF, tag="xth", name=f"xth{b}_{h2}", bufs=B_ * HPC
                            )
                            nc.sync.dma_start(
                                out=xth[h2][:, 0 : S_ // 2], in_=xt[b, h2, :, 0 : S_ // 2]
                            )
                            nc.sync.dma_start(
                                out=xth[h2][:, S_ // 2 : S_], in_=xt[b, h2, :, S_ // 2 : S_]
                            )
                    xs_b = sb.tile([128, S_], F32, tag="xs", name=f"xs{b}")
                    y_b = sb.tile([128, S_], F32, tag=f"y{b}", name=f"y{b}")
                    y_tiles[b] = y_b
                    bstate[b] = (xth, xs_b, y_b, {})
                    need_xs_dma = True
                else:
                    need_xs_dma = False
                xth, xs_b, y_b, accs = bstate[b]
                xh = xth[hh]
                # z = [M @ xh^T + u | beta-row]: scores become xh_aug^T @ z
                z = sb.tile([DK + 1, S_], BF, tag="z", name=f"z{pair}", bufs=NP)
                for c in range(S_ // 512):
                    zp = ps.tile([128, 512], F32, tag="op", bufs=2, name=f"zp{c}")
                    nc.tensor.matmul(
                        zp[0 : DK + 1, :],
                        lhsT=zw_t[:, (DK + 1) * hh : (DK + 1) * (hh + 1)],
                        rhs=xh[0:DK, 512 * c : 512 * c + 512],
                        start=True,
                        stop=True,
                    )
                    nc.vector.tensor_scalar(
                        z[:, 512 * c : 512 * c + 512],
                        zp[0 : DK + 1, :],
                        zb_t[:, hh : hh + 1],
                        None,
                        A.add,
                    )
                # V with bias, ones-augmented: v = [V | 1] blocks of 65 cols
                v = sb.tile([128, NT_ * (DK + 1)], BF, tag="v", name=f"v{pair}", bufs=NP)
                v3 = v[:].rearrange("p (t w) -> p t w", w=DK + 1)
                nc.vector.memset(v3[:, :, DK : DK + 1], 1.0)
                gv = min(8, NT_)
                for g in range(NT_ // gv):
                    vp = ps.tile([128, 512], F32, tag="op", bufs=2, name=f"vp{g}")
                    for u in range(gv):
                        j = gv * g + u
                        nc.tensor.matmul(
                            vp[:, DK * u : DK * u + DK],
                            lhsT=xh[0:DK, 128 * j : 128 * j + 128],
                            rhs=wv_t[:, hh * DK : hh * DK + DK],
                            start=True,
                            stop=True,
                        )
                    nc.vector.tensor_tensor(
                        v3[:, gv * g : gv * g + gv, 0:DK],
                        vp[:, 0 : gv * DK].rearrange("q (t w) -> q t w", w=DK),
                        bv16_t[:].rearrange("q (h t w) -> q (h t) w", h=HPC, w=DK)[
                            :, hh * 16 : hh * 16 + gv, :
                        ],
                        A.add,
                    )
                if need_xs_dma:
                    nc.sync.dma_start(
                        out=xs_b[:].rearrange("p (i d) -> p i d", d=128),
                        in_=xs[b].rearrange("(i p) d -> p i d", p=128),
                    )
                acc_h = sb.tile([128, NT_], F32, tag=f"acc{hh}", name=f"acc{pair}", bufs=B_)
                accs[hh] = acc_h
                pstate[pair] = (xh, z, v3, acc_h)

            def emit_jhalf(pair, hs, mid_hook=None):
                """Score/exp/PV loop for one 1024-col s-half; returns the
                deferred transpose/normalize epilogue closure."""
                b, hh = divmod(pair, HPC)
                xh, z, v3, acc_h = pstate[pair]
                _, xs_b, y_b, accs = bstate[b]
                he = min(S_, hs + 1024)
                w = he - hs
                opA = ps.tile([DK + 1, 512], F32, tag="op", bufs=2)
                opB = ps.tile([DK + 1, 512], F32, tag="op", bufs=2)
                prev_pv = None
                for j in range(he // 128):
                    s0 = 128 * j
                    rel = s0 - hs
                    p = sb.tile([128, 1024], BF, tag="p", bufs=PBUFS)
                    sp = ps.tile([128, 1024], F32, tag="sp", bufs=SPBUFS)
                    if rel < 0:
                        ss = 0
                        while ss < w:
                            sl = min(512, w - ss)
                            nc.tensor.matmul(
                                sp[:, ss : ss + sl],
                                lhsT=xh[:, s0 : s0 + 128],
                                rhs=z[:, hs + ss : hs + ss + sl],
                                start=True,
                                stop=True,
                            )
                            ss += sl
                        lo = 0
                    else:
                        lo = rel
                        nc.tensor.matmul(
                            sp[:, rel : rel + 128],
                            lhsT=idn128_t,
                            rhs=maskt_t,
                            start=True,
                            stop=False,
                            skip_group_check=True,
                        )
                        nc.tensor.matmul(
                            sp[:, rel : rel + 128],
                            lhsT=xh[:, s0 : s0 + 128],
                            rhs=z[:, s0 : s0 + 128],
                            start=False,
                            stop=True,
                            skip_group_check=True,
                        )
                        ss = rel + 128
                        while ss < w:
                            sl = min(512 - (ss % 512), w - ss)
                            nc.tensor.matmul(
                                sp[:, ss : ss + sl],
                                lhsT=xh[:, s0 : s0 + 128],
                                rhs=z[:, hs + ss : hs + ss + sl],
                                start=True,
                                stop=True,
                            )
                            ss += sl
                    nc.scalar.activation(p[:, lo:w], sp[:, lo:w], AF.Exp)

                    # PV deferred by one j so PE computes S_{j+1} while the
                    # ACT engine exps j (avoids PE stalling on exp latency)
                    def _pv(j=j, p=p, lo=lo):
                        cs = lo
                        while cs < w:
                            ce = min(512 * (cs // 512) + 512, w)
                            gc = (hs + cs) // 512
                            opt = opA if cs < 512 else opB
                            nc.tensor.matmul(
                                opt[:, cs % 512 : cs % 512 + (ce - cs)],
                                lhsT=v3[:, j, :],
                                rhs=p[:, cs:ce],
                                start=(j == 0),
                                stop=(j == min(he // 128 - 1, 4 * gc + 3)),
                                skip_group_check=True,
                            )
                            cs = ce

                    if prev_pv is not None:
                        prev_pv()
                    prev_pv = _pv
                    if j == min(HOOKJ, he // 128 - 1) and mid_hook is not None:
                        mid_hook()
                if prev_pv is not None:
                    prev_pv()
                # drain O^T; transpose/normalize deferred
                ot = sb.tile([DK + 1, 1024], F32, tag="ot", bufs=2 + EPI_LAG)
                nc.vector.tensor_copy(ot[:, 0 : min(512, w)], opA[:, 0 : min(512, w)])
                if w > 512:
                    nc.vector.tensor_copy(ot[:, 512:w], opB[:, 0 : w - 512])

                def _epilogue():
                    nk = he // 128 - hs // 128
                    # transposes staged in two 1-bank tiles on the op ring
                    # (freed by the early accumulator drains), keeping the
                    # score ring untouched by the epilogue
                    tps = [
                        ps.tile([128, 512], F32, tag="op", bufs=2, name=f"tp{g}")
                        for g in range((nk + 3) // 4)
                    ]
                    for i in range(hs // 128, he // 128):
                        k = i - hs // 128
                        tp = tps[k // 4]
                        nc.tensor.transpose(
                            tp[:, 128 * (k % 4) : 128 * (k % 4) + DK + 1],
                            ot[:, 128 * i - hs : 128 * i - hs + 128],
                            idn_t[:],
                        )
                    r8 = sb.tile([128, 8], F32, tag="r8", bufs=3)
                    for g, tp in enumerate(tps):
                        gn = min(4, nk - 4 * g)
                        nc.vector.reciprocal(
                            r8[:, 4 * g : 4 * g + gn],
                            tp[:].rearrange("q (k c) -> q k c", c=128)[
                                :, 0:gn, DK : DK + 1
                            ],
                        )
                    for i in range(hs // 128, he // 128):
                        k = i - hs // 128
                        tp = tps[k // 4]
                        nc.vector.scalar_tensor_tensor(
                            y_b[:, 128 * i + DK * hh : 128 * i + DK * hh + DK],
                            tp[:, 128 * (k % 4) : 128 * (k % 4) + DK],
                            r8[:, k : k + 1],
                            xs_b[:, 128 * i + DK * hh : 128 * i + DK * hh + DK],
                            A.mult,
                            A.add,
                            accum_out=acc_h[:, i : i + 1],
                        )
                    if hh == HPC - 1:
                        _emit_stats(b, y_b, accs, hs, he, hs // 1024)

                return _epilogue

            emit_proj(0)
            pending = []
            for pair in range(NP):
                for k, hs in enumerate(range(0, S_, 1024)):
                    hook = None
                    if k == 0 and pair + 1 < NP:
                        hook = (lambda pr=pair: emit_proj(pr + 1))
                    epi = emit_jhalf(pair, hs, mid_hook=hook)
                    pending.append(epi)
                    if len(pending) > EPI_LAG:
                        pending.pop(0)()
            for e in pending:
                e()

            for b in range(B_):
                for ch in range(NHALF):
                    emit_ln(b, ch, y_tiles[b])


    # Restrict Exp/Ln to the shared natural_log_exp_and_others table set so
    # the whole kernel uses one ACT table load (indices preserved).
    import concourse.bacc as _bacc_mod

    _orig_tables = _bacc_mod.get_activation_tables

    def _filtered_tables(arch):
        out = {}
        for name, fns in _orig_tables(arch).items():
            if name != "natural_log_exp_and_others":
                fns = set(fns) - {AF.Exp, AF.Ln}
            out[name] = fns
        return out

    _bacc_mod.get_activation_tables = _filtered_tables
    try:
        nc.compile()
    finally:
        _bacc_mod.get_activation_tables = _orig_tables
    return nc


_GRAPH_CACHE = {}


def _get_graph(apply_affine: bool) -> bass.Bass:
    if apply_affine not in _GRAPH_CACHE:
        _GRAPH_CACHE[apply_affine] = _build_graph(apply_affine)
    return _GRAPH_CACHE[apply_affine]


def _prep_in_maps(x, Wq, bq, Wk, bk, Wv, bv, gamma, beta, apply_affine):
    scale = 1.0 / np.sqrt(np.float32(DK))
    in_maps = []
    for i in range(NCORES):
        dsl = slice(DC * i, DC * (i + 1))
        hsl = slice(HPC * i, HPC * (i + 1))
        x_sl = x[:, :, dsl]
        xt_full = x_sl.transpose(0, 2, 1).reshape(x.shape[0], HPC, DK, x.shape[1])
        xt_aug = np.concatenate(
            [xt_full, np.ones((x.shape[0], HPC, 1, x.shape[1]), np.float32)], axis=2
        )
        Wq_s = (Wq[hsl] * scale).astype(np.float64)
        bq_s = (bq[hsl] * scale).astype(np.float64)
        Wk_h = Wk[hsl].astype(np.float64)
        bk_h = bk[hsl].astype(np.float64)
        M = np.einsum("hde,hfe->hdf", Wk_h, Wq_s)      # [h, dK, dQ]
        u = np.einsum("hde,he->hd", Wk_h, bq_s)        # alpha coeffs (per t)
        wvec = np.einsum("hde,he->hd", Wq_s, bk_h)     # beta coeffs (per s)
        cconst = np.einsum("he,he->h", bk_h, bq_s)
        # lhsT for z: [d', dK | wvec]; z rows 0..63 = M@xh^T + u, row 64 = xh.w + c
        zw_np = np.concatenate(
            [M.transpose(0, 2, 1), wvec[:, :, None]], axis=2
        )  # [h, d'(=dQ... contraction dim), dK+1]
        zb_np = np.concatenate([u, cconst[:, None]], axis=1)[:, :, None]
        m = {
            "xt": np.ascontiguousarray(xt_aug).astype(BF_NP),
            "xs": np.ascontiguousarray(x_sl),
            "wpack": np.ascontiguousarray(
                np.concatenate(
                    [zw_np[0], zw_np[1], Wv[hsl][0], Wv[hsl][1]], axis=1
                )
            ).astype(BF_NP),
            "zb": np.ascontiguousarray(zb_np[:, :, 0].T).astype(np.float32),
            "bv16": np.ascontiguousarray(
                np.tile(bv[hsl][:, None, :], (1, 128, 16))
            ).astype(np.float32),
        }
        if apply_affine:
            m["gam"] = np.ascontiguousarray(
                np.tile(gamma[dsl][None, :], (128, 1))
            ).astype(np.float32)
            m["bet"] = np.ascontiguousarray(
                np.tile(beta[dsl][None, :], (128, 1))
            ).astype(np.float32)
        in_maps.append(m)
    return in_maps


def kernel(x, Wq, bq, Wk, bk, Wv, bv, gamma, beta):
    global LAST_RESULTS
    x = np.asarray(x, np.float32)
    Wq = np.asarray(Wq, np.float32)
    bq = np.asarray(bq, np.float32)
    Wk = np.asarray(Wk, np.float32)
    bk = np.asarray(bk, np.float32)
    Wv = np.asarray(Wv, np.float32)
    bv = np.asarray(bv, np.float32)
    gamma = np.asarray(gamma, np.float32)
    beta = np.asarray(beta, np.float32)

    apply_affine = not (
        np.allclose(gamma, 1.0, atol=0.0, rtol=0.0)
        and np.allclose(beta, 0.0, atol=0.0, rtol=0.0)
    )
    fake_ar = bool(int(os.environ.get("KERNEL_FAKE_AR", "0")))
    nc = _get_graph(apply_affine) if not fake_ar else _build_graph(apply_affine, fake_ar=True)

    in_maps = _prep_in_maps(x, Wq, bq, Wk, bk, Wv, bv, gamma, beta, apply_affine)

    res = run_bass_kernel_spmd(
        nc,
        in_maps,
        core_ids=list(range(NCORES)),
        trace=bool(int(os.environ.get("KERNEL_TRACE", "0"))),
    )
    LAST_RESULTS = res
    outs = [np.asarray(r["out"], np.float32) for r in res.results]
    return np.concatenate(outs, axis=2)


if __name__ == "__main__":
    nc = _build_graph(False)
    print("graph built ok:", len(nc.inst_map), "instructions")



# revision 8
# speedup vs baseline: 1.0396x; 1.0396x over previous
"""Trainium2 Bass kernel: per-head attention + residual + LayerNorm.

Problem shape: x [4, 2048, 1024], 16 heads of dk=64, causal softmax attention
with per-head Q/K/V linear projections, residual add, LayerNorm(D).

Sharding (8 cores): head-parallel. Core i owns heads (2i, 2i+1), i.e. feature
columns 128*i : 128*(i+1). Each core computes its feature slice of the output;
the only cross-core communication is a tiny per-batch AllReduce of LayerNorm
partial sums (sum and sum-of-squares over each core's 128 features). The host
shards/gathers and pre-transposes x (the [dk, S] layout each head needs).

Per-core kernel design (bf16 matmuls, fp32 PSUM accumulation):
- Scores via a host-precomputed bilinear form: scores^T = xh_aug^T @ z with
  z = [M @ xh^T + u | beta-row], M = Wk (Wq/sqrt(dk))^T. One projected tensor
  (z) instead of Q and K halves the PSUM->SBUF copies; x^T itself (with a
  built-in ones row for the bias terms) is the stationary matmul operand.
- Flash-style t-outer loop over 1024-col query halves; scores accumulate in
  PSUM [128,1024] chunks on a dedicated 3-deep ring (6 banks) that serves
  ONLY the score->exp pipeline; exp runs on ScalarE straight from PSUM in
  one instruction per (t-block, half), P is bf16 in SBUF. All other PSUM
  users (split O^T accumulators, projection staging, epilogue transposes)
  live on a separate ring of fast-release 1-bank [*,512] slots (2 banks),
  so no phase ever blocks the score pipeline's buffers.
- Causal mask: an identity-matmul accumulates -40 onto the upper triangle of
  the diagonal 128-block before exp (no vector-engine masking); sub-diagonal
  dead zones are simply never read by PV.
- PV accumulates O^T [65,1024] in PSUM with a ones-augmented V, so softmax
  denominators ride along as row 64; per 128-tile PE-transposes then let a
  single fused DVE op do (O*1/l + x) with the row-sum accumulated for free.
- rstd = exp(-0.5*ln(var+eps)) keeps every activation in one ACT table set
  (natural_log_exp_and_others; enforced by filtering the set map at compile).
- Emission is software-pipelined: the next pair's z/V projection is emitted
  mid-way through the current score loop, the transpose/normalize epilogue is
  deferred one unit, and per-half LayerNorm stats AllReduce in 8 small chunks
  so TensorE/ScalarE/VectorE and the collective overlap across units.

Self-contained: hardcodes all shapes; no sibling imports.
"""

import os
import numpy as np
import ml_dtypes

import concourse.bass as bass
import concourse.bacc as bacc
import concourse.mybir as mybir
from concourse.tile import TileContext
from concourse.bass_utils import run_bass_kernel_spmd

B, S, D, H = 4, 2048, 1024, 16
NCORES = 8
HPC = H // NCORES          # heads per core = 2
DK = D // H                # 64
DC = HPC * DK              # 128 feature cols per core
NT = S // 128              # 16 row tiles of 128
EPS = 1e-5
MASKNEG = -40.0
SPBUFS = 3
OPBUFS = 1
EPI_LAG = 1
HOOKJ = 6
QKBUFS = 3
PBUFS = 3
BF = mybir.dt.bfloat16
F32 = mybir.dt.float32
FP8 = mybir.dt.float8e4
FP8E5 = mybir.dt.float8e5
BF_NP = ml_dtypes.bfloat16
FP8_NP = ml_dtypes.float8_e4m3
FP8E5_NP = ml_dtypes.float8_e5m2
VW = 80  # padded V-tile row stride (65 used) so DoubleRow plane step is 16B-aligned
RG = [list(range(NCORES))]
A = mybir.AluOpType
AF = mybir.ActivationFunctionType

LAST_RESULTS = None  # BassKernelResults of the last run (for test harness)


def _build_graph(apply_affine: bool, B_: int = B, S_: int = S, rg=None, fake_ar: bool = False) -> bass.Bass:
    nc = bacc.Bacc()
    NT_ = S_ // 128
    if rg is None:
        rg = RG

    xt = nc.declare_dram_parameter("xt", [B_, HPC, DK + 1, S_], BF, isOutput=False)
    xs = nc.declare_dram_parameter("xs", [B_, S_, DC], F32, isOutput=False)
    wpack = nc.declare_dram_parameter(
        "wpack", [DK, HPC * (DK + 1) + HPC * DK], BF, isOutput=False
    )
    zb = nc.declare_dram_parameter("zb", [DK + 1, HPC], F32, isOutput=False)
    bv16 = nc.declare_dram_parameter("bv16", [HPC, 128, 16 * DK], F32, isOutput=False)
    if apply_affine:
        gam = nc.declare_dram_parameter("gam", [128, DC], F32, isOutput=False)
        bet = nc.declare_dram_parameter("bet", [128, DC], F32, isOutput=False)
    out = nc.declare_dram_parameter("out", [B_, S_, DC], F32, isOutput=True)

    # constants baked into the NEFF
    idn_h = nc.inline_tensor(np.eye(DK + 1, dtype=np.float32), name="idn")
    trineg_np = np.where(
        np.arange(128)[:, None] > np.arange(128)[None, :], MASKNEG, 0.0
    ).astype(np.float32)
    # DoubleRow-plane layout: [64, 2, 128] -> [64, 256], plane i = rows 64i..64i+63
    idn2_np = np.eye(128, dtype=np.float32).reshape(2, 64, 128).transpose(1, 0, 2)
    trineg2_np = trineg_np.reshape(2, 64, 128).transpose(1, 0, 2)
    imask_h = nc.inline_tensor(
        np.concatenate([idn2_np, trineg2_np], axis=2).reshape(64, 512).astype(
            FP8E5_NP
        ),
        name="imask",
    )

    # collective bounce buffers: LayerNorm stats per (batch, s-half):
    # [2(sum,sumsq), 128 rows, tiles-in-half]
    NHALF = (S_ + 1023) // 1024
    NTH = NT_ // NHALF
    stats_in = nc.dram_tensor("stats_in", [B_, NHALF, 2, 128, NTH], F32)
    stats_out = nc.dram_tensor(
        "stats_out", [B_, NHALF, 2, 128, NTH], F32, addr_space="Shared"
    )

    with TileContext(nc) as tc:
        with (
            tc.tile_pool(name="consts", bufs=1) as cpool,
            tc.tile_pool(name="sb", bufs=2) as sb,
            tc.tile_pool(name="ps", bufs=1, space="PSUM") as ps,
        ):
            # ---- load constants (single coalesced DMAs, first-needed first) ----
            wp_t = cpool.tile([DK, HPC * (DK + 1) + HPC * DK], BF, tag="wp")
            nc.sync.dma_start(out=wp_t[:], in_=wpack[:, :])
            zw_t = wp_t[:][:, 0 : HPC * (DK + 1)]
            wv_t = wp_t[:][:, HPC * (DK + 1) : HPC * (DK + 1) + HPC * DK]
            # queue order tuned for the first exp: wpack, then the first
            # half of head-0 x^T (all the first projection needs), then the
            # small constants the first z-copy and diagonal mask need, then
            # the rest of batch-0 x^T
            xth0 = [
                sb.tile([DK + 1, S_], BF, tag="xth", name=f"xth0_{h2}", bufs=B_ * HPC)
                for h2 in range(HPC)
            ]
            nc.sync.dma_start(out=xth0[0][:, 0 : S_ // 2], in_=xt[0, 0, :, 0 : S_ // 2])
            zbq_t = cpool.tile([DK + 1, HPC], F32, tag="zb")
            nc.sync.dma_start(out=zbq_t[:], in_=zb[:, :])
            imaskq_t = cpool.tile([64, 512], FP8E5, tag="imask")
            nc.sync.dma_start(out=imaskq_t[:], in_=imask_h[:, :])
            nc.sync.dma_start(out=xth0[0][:, S_ // 2 : S_], in_=xt[0, 0, :, S_ // 2 : S_])
            nc.sync.dma_start(out=xth0[1][:, 0 : S_ // 2], in_=xt[0, 1, :, 0 : S_ // 2])
            nc.sync.dma_start(out=xth0[1][:, S_ // 2 : S_], in_=xt[0, 1, :, S_ // 2 : S_])
            idn_t = cpool.tile([DK + 1, DK + 1], F32, tag="idn")
            nc.gpsimd.dma_start(out=idn_t[:], in_=idn_h[:, :])
            bv16_t = cpool.tile([128, HPC * 16 * DK], F32, tag="bv16")
            for h in range(HPC):
                nc.gpsimd.dma_start(
                    out=bv16_t[:, 16 * DK * h : 16 * DK * (h + 1)], in_=bv16[h]
                )
            if apply_affine:
                gam_t = cpool.tile([128, DC], F32, tag="gam")
                nc.sync.dma_start(out=gam_t[:], in_=gam[:, :])
                bet_t = cpool.tile([128, DC], F32, tag="bet")
                nc.sync.dma_start(out=bet_t[:], in_=bet[:, :])

            zb_t = zbq_t
            imask3 = imaskq_t[:].rearrange("p (i c) -> p i c", c=256)
            idn128_t = imask3[:, :, 0:128]
            maskt_t = imask3[:, :, 128:256]
            eps_t = cpool.tile([128, 1], F32, tag="eps")
            nc.vector.memset(eps_t[:], EPS)

            pending_epi = [None]

            def _emit_stats(b, y_b, acc, hs, he, ch):
                # LayerNorm partial stats + AllReduce for one s-half
                t0, t1 = hs // 128, he // 128
                nth = t1 - t0
                sums = sb.tile([128, NTH], F32, tag="sums", bufs=3)
                nc.vector.tensor_add(
                    sums[:, 0:nth], acc[0][:, t0:t1], acc[1][:, t0:t1]
                )
                sq = sb.tile([128, NTH], F32, tag="sq", bufs=3)
                for i in range(t0, t1):
                    scr = sb.tile([128, 128], F32, tag="scr")
                    nc.vector.scalar_tensor_tensor(
                        scr[:],
                        y_b[:, 128 * i : 128 * i + 128],
                        1.0,
                        y_b[:, 128 * i : 128 * i + 128],
                        A.mult,
                        A.mult,
                        accum_out=sq[:, i - t0 : i - t0 + 1],
                    )
                nc.sync.dma_start(out=stats_in[b, ch, 0], in_=sums[:, 0:nth])
                nc.sync.dma_start(out=stats_in[b, ch, 1], in_=sq[:, 0:nth])
                if fake_ar:
                    nc.sync.dma_start(out=stats_out[b, ch], in_=stats_in[b, ch])
                else:
                    nc.gpsimd.collective_compute(
                        "AllReduce",
                        A.add,
                        replica_groups=rg,
                        ins=[stats_in[b, ch].opt()],
                        outs=[stats_out[b, ch].opt()],
                    )

            def emit_ln(b, ch, y_b):
                t0 = ch * NTH
                red = sb.tile([128, 2 * NTH], F32, tag="red", bufs=3)
                nc.sync.dma_start(
                    out=red[:].rearrange("p (c t) -> p c t", t=NTH),
                    in_=stats_out[b, ch].rearrange("c p t -> p c t"),
                )
                mean = sb.tile([128, NTH], F32, tag="mean", bufs=3)
                nc.vector.tensor_scalar(
                    mean[:], red[:, 0:NTH], 1.0 / D, None, A.mult
                )
                msq = sb.tile([128, NTH], F32, tag="msq", bufs=3)
                nc.vector.tensor_mul(msq[:], mean[:], mean[:])
                var = sb.tile([128, NTH], F32, tag="var", bufs=3)
                nc.vector.scalar_tensor_tensor(
                    var[:], red[:, NTH : 2 * NTH], 1.0 / D, msq[:], A.mult,
                    A.subtract,
                )
                lnv = sb.tile([128, NTH], F32, tag="lnv", bufs=3)
                nc.scalar.activation(lnv[:], var[:], AF.Ln, bias=eps_t[:])
                rstd = sb.tile([128, NTH], F32, tag="rstd", bufs=3)
                nc.scalar.activation(rstd[:], lnv[:], AF.Exp, scale=-0.5)
                ostb = sb.tile([128, 128 * NTH], F32, tag="ost", bufs=2)
                for k in range(NTH):
                    i = t0 + k
                    nc.vector.tensor_scalar(
                        ostb[:, 128 * k : 128 * k + 128],
                        y_b[:, 128 * i : 128 * i + 128],
                        mean[:, k : k + 1],
                        rstd[:, k : k + 1],
                        A.subtract,
                        A.mult,
                    )
                    if apply_affine:
                        nc.vector.tensor_mul(
                            ostb[:, 128 * k : 128 * k + 128],
                            ostb[:, 128 * k : 128 * k + 128],
                            gam_t[:],
                        )
                        nc.vector.tensor_add(
                            ostb[:, 128 * k : 128 * k + 128],
                            ostb[:, 128 * k : 128 * k + 128],
                            bet_t[:],
                        )
                eng = nc.gpsimd if ((b * NHALF + ch) % 2 == 0 and b < B_ - 1) else nc.sync
                eng.dma_start(
                    out=out[b, 128 * t0 : 128 * (t0 + NTH), :].rearrange(
                        "(i p) d -> p i d", p=128
                    ),
                    in_=ostb[:].rearrange("p (i d) -> p i d", d=128),
                )

            y_tiles = {}
            bstate = {}
            pstate = {}
            pw = min(1024, S_)
            NP = B_ * HPC

            def emit_proj(pair):
                b, hh = divmod(pair, HPC)
                if hh == 0:
                    if b == 0:
                        xth = xth0
                    else:
                        xth = [None, None]
                        for h2 in range(HPC):
                            xth[h2] = sb.tile(
                                [DK + 1, S_], BF, tag="xth", name=f"xth{b}_{h2}", bufs=B_ * HPC
                            )
                            nc.sync.dma_start(
                                out=xth[h2][:, 0 : S_ // 2], in_=xt[b, h2, :, 0 : S_ // 2]
                            )
                            nc.sync.dma_start(
                                out=xth[h2][:, S_ // 2 : S_], in_=xt[b, h2, :, S_ // 2 : S_]
                            )
                    xs_b = sb.tile([128, S_], F32, tag="xs", name=f"xs{b}")
                    y_b = sb.tile([128, S_], F32, tag=f"y{b}", name=f"y{b}")
                    y_tiles[b] = y_b
                    bstate[b] = (xth, xs_b, y_b, {})
                    need_xs_dma = True
                else:
                    need_xs_dma = False
                xth, xs_b, y_b, accs = bstate[b]
                xh = xth[hh]
                # z = [M @ xh^T + u | beta-row]: scores become xh_aug^T @ z
                z = sb.tile([DK + 1, S_], BF, tag="z", name=f"z{pair}", bufs=NP)
                for c in range(S_ // 512):
                    zp = ps.tile([128, 512], F32, tag="op", bufs=2, name=f"zp{c}")
                    nc.tensor.matmul(
                        zp[0 : DK + 1, :],
                        lhsT=zw_t[:, (DK + 1) * hh : (DK + 1) * (hh + 1)],
                        rhs=xh[0:DK, 512 * c : 512 * c + 512],
                        start=True,
                        stop=True,
                    )
                    nc.vector.tensor_scalar(
                        z[:, 512 * c : 512 * c + 512],
                        zp[0 : DK + 1, :],
                        zb_t[:, hh : hh + 1],
                        None,
                        A.add,
                    )
                # V with bias, ones-augmented: v = [V | 1] blocks, fp8 for the
                # DoubleRow PV; VW-padded so plane stride is 16B-aligned
                v = sb.tile([128, NT_ * VW], FP8, tag="v", name=f"v{pair}", bufs=NP)
                v3 = v[:].rearrange("p (t w) -> p t w", w=VW)
                nc.vector.memset(v3[:, :, DK : DK + 1], 1.0)
                gv = min(8, NT_)
                for g in range(NT_ // gv):
                    vp = ps.tile([128, 512], F32, tag="op", bufs=2, name=f"vp{g}")
                    for u in range(gv):
                        j = gv * g + u
                        nc.tensor.matmul(
                            vp[:, DK * u : DK * u + DK],
                            lhsT=xh[0:DK, 128 * j : 128 * j + 128],
                            rhs=wv_t[:, hh * DK : hh * DK + DK],
                            start=True,
                            stop=True,
                        )
                    nc.vector.tensor_tensor(
                        v3[:, gv * g : gv * g + gv, 0:DK],
                        vp[:, 0 : gv * DK].rearrange("q (t w) -> q t w", w=DK),
                        bv16_t[:].rearrange("q (h t w) -> q (h t) w", h=HPC, w=DK)[
                            :, hh * 16 : hh * 16 + gv, :
                        ],
                        A.add,
                    )
                if need_xs_dma:
                    nc.sync.dma_start(
                        out=xs_b[:].rearrange("p (i d) -> p i d", d=128),
                        in_=xs[b].rearrange("(i p) d -> p i d", p=128),
                    )
                acc_h = sb.tile([128, NT_], F32, tag=f"acc{hh}", name=f"acc{pair}", bufs=B_)
                accs[hh] = acc_h
                pstate[pair] = (xh, z, v3, acc_h)

            def emit_jhalf(pair, hs, mid_hook=None):
                """Score/exp/PV loop for one 1024-col s-half; returns the
                deferred transpose/normalize epilogue closure. PV runs as
                fp8 DoubleRow over PAIRS of key-tiles (j, j+1): V planes are
                adjacent slices of v3, P planes are halves of a shared fp8
                tile — one matmul covers two key-tiles at 0.5 cycles/col."""
                b, hh = divmod(pair, HPC)
                xh, z, v3, acc_h = pstate[pair]
                _, xs_b, y_b, accs = bstate[b]
                he = min(S_, hs + 1024)
                w = he - hs
                nj = he // 128
                npair_t = nj // 2
                opA = ps.tile([DK + 1, 512], F32, tag="op", bufs=2)
                opB = ps.tile([DK + 1, 512], F32, tag="op", bufs=2)
                # last tile-pair touching each 512-col bank (for stop flags)
                last_m = [
                    max(
                        mm
                        for mm in range(npair_t)
                        if max(0, 256 * mm - hs) < 512 * (g + 1)
                    )
                    for g in range(w // 512)
                ]
                prev_pv = None
                for m in range(npair_t):
                    p2m = sb.tile(
                        [128, 2048], FP8, tag="p", bufs=PBUFS, name=f"p2_{m}"
                    )
                    p3 = p2m[:].rearrange("q (i c) -> q i c", c=1024)
                    los = [0, 0]
                    for par in range(2):
                        j = 2 * m + par
                        s0 = 128 * j
                        rel = s0 - hs
                        sp = ps.tile([128, 1024], F32, tag="sp", bufs=SPBUFS)
                        if rel < 0:
                            ss = 0
                            while ss < w:
                                sl = min(512, w - ss)
                                nc.tensor.matmul(
                                    sp[:, ss : ss + sl],
                                    lhsT=xh[:, s0 : s0 + 128],
                                    rhs=z[:, hs + ss : hs + ss + sl],
                                    start=True,
                                    stop=True,
                                )
                                ss += sl
                            lo = 0
                        else:
                            lo = rel
                            nc.tensor.matmul(
                                sp[:, rel : rel + 128],
                                lhsT=idn128_t,
                                rhs=maskt_t,
                                start=True,
                                stop=False,
                                perf_mode=mybir.MatmulPerfMode.DoubleRow,
                                skip_group_check=True,
                            )
                            nc.tensor.matmul(
                                sp[:, rel : rel + 128],
                                lhsT=xh[:, s0 : s0 + 128],
                                rhs=z[:, s0 : s0 + 128],
                                start=False,
                                stop=True,
                                skip_group_check=True,
                            )
                            ss = rel + 128
                            while ss < w:
                                sl = min(512 - (ss % 512), w - ss)
                                nc.tensor.matmul(
                                    sp[:, ss : ss + sl],
                                    lhsT=xh[:, s0 : s0 + 128],
                                    rhs=z[:, hs + ss : hs + ss + sl],
                                    start=True,
                                    stop=True,
                                )
                                ss += sl
                        los[par] = lo
                        nc.scalar.activation(p3[:, par, lo:w], sp[:, lo:w], AF.Exp)
                        if j == min(HOOKJ, nj - 1) and mid_hook is not None:
                            mid_hook()

                    # PV for the pair, deferred by one pair so the PE computes
                    # the next scores while ACT exps this pair
                    def _pv(m=m, p3=p3, lo0=los[0], lo1=los[1]):
                        j0 = 2 * m
                        # ragged diagonal: [lo0, lo1) has only plane 0 valid
                        cs = lo0
                        while cs < lo1:
                            ce = min(512 * (cs // 512) + 512, lo1)
                            opt = opA if cs < 512 else opB
                            nc.tensor.matmul(
                                opt[:, cs % 512 : cs % 512 + (ce - cs)],
                                lhsT=v3[:, j0, 0 : DK + 1],
                                rhs=p3[:, 0, cs:ce],
                                start=(m == 0),
                                stop=False,
                                skip_group_check=True,
                            )
                            cs = ce
                        # both planes valid: DoubleRow over (j0, j0+1)
                        cs = lo1
                        while cs < w:
                            ce = min(512 * (cs // 512) + 512, w)
                            g = cs // 512
                            opt = opA if cs < 512 else opB
                            first = (m == 0) and (cs == lo1 == lo0 or cs >= 512)
                            nc.tensor.matmul(
                                opt[:, cs % 512 : cs % 512 + (ce - cs)],
                                lhsT=v3[:, j0 : j0 + 2, 0 : DK + 1],
                                rhs=p3[:, :, cs:ce],
                                start=first,
                                stop=(m == last_m[g] and ce == min(w, 512 * (g + 1))),
                                perf_mode=mybir.MatmulPerfMode.DoubleRow,
                                skip_group_check=True,
                            )
                            cs = ce

                    if prev_pv is not None:
                        prev_pv()
                    prev_pv = _pv
                if prev_pv is not None:
                    prev_pv()
                # drain O^T; transpose/normalize deferred
                ot = sb.tile([DK + 1, 1024], F32, tag="ot", bufs=2 + EPI_LAG)
                nc.vector.tensor_copy(ot[:, 0 : min(512, w)], opA[:, 0 : min(512, w)])
                if w > 512:
                    nc.vector.tensor_copy(ot[:, 512:w], opB[:, 0 : w - 512])

                def _epilogue():
                    nk = he // 128 - hs // 128
                    # transposes staged in two 1-bank tiles on the op ring
                    # (freed by the early accumulator drains), keeping the
                    # score ring untouched by the epilogue
                    tps = [
                        ps.tile([128, 512], F32, tag="op", bufs=2, name=f"tp{g}")
                        for g in range((nk + 3) // 4)
                    ]
                    for i in range(hs // 128, he // 128):
                        k = i - hs // 128
                        tp = tps[k // 4]
                        nc.tensor.transpose(
                            tp[:, 128 * (k % 4) : 128 * (k % 4) + DK + 1],
                            ot[:, 128 * i - hs : 128 * i - hs + 128],
                            idn_t[:],
                        )
                    r8 = sb.tile([128, 8], F32, tag="r8", bufs=3)
                    for g, tp in enumerate(tps):
                        gn = min(4, nk - 4 * g)
                        nc.vector.reciprocal(
                            r8[:, 4 * g : 4 * g + gn],
                            tp[:].rearrange("q (k c) -> q k c", c=128)[
                                :, 0:gn, DK : DK + 1
                            ],
                        )
                    for i in range(hs // 128, he // 128):
                        k = i - hs // 128
                        tp = tps[k // 4]
                        nc.vector.scalar_tensor_tensor(
                            y_b[:, 128 * i + DK * hh : 128 * i + DK * hh + DK],
                            tp[:, 128 * (k % 4) : 128 * (k % 4) + DK],
                            r8[:, k : k + 1],
                            xs_b[:, 128 * i + DK * hh : 128 * i + DK * hh + DK],
                            A.mult,
                            A.add,
                            accum_out=acc_h[:, i : i + 1],
                        )
                    if hh == HPC - 1:
                        _emit_stats(b, y_b, accs, hs, he, hs // 1024)

                return _epilogue

            emit_proj(0)
            pending = []
            for pair in range(NP):
                for k, hs in enumerate(range(0, S_, 1024)):
                    hook = None
                    if k == 0 and pair + 1 < NP:
                        hook = (lambda pr=pair: emit_proj(pr + 1))
                    epi = emit_jhalf(pair, hs, mid_hook=hook)
                    pending.append(epi)
                    if len(pending) > EPI_LAG:
                        pending.pop(0)()
            for e in pending:
                e()

            for b in range(B_):
                for ch in range(NHALF):
                    emit_ln(b, ch, y_tiles[b])


    # Restrict Exp/Ln to the shared natural_log_exp_and_others table set so
    # the whole kernel uses one ACT table load (indices preserved).
    import concourse.bacc as _bacc_mod

    _orig_tables = _bacc_mod.get_activation_tables

    def _filtered_tables(arch):
        out = {}
        for name, fns in _orig_tables(arch).items():
            if name != "natural_log_exp_and_others":
                fns = set(fns) - {AF.Exp, AF.Ln}
            out[name] = fns
        return out

    _bacc_mod.get_activation_tables = _filtered_tables
    try:
        nc.compile()
    finally:
        _bacc_mod.get_activation_tables = _orig_tables
    return nc


_GRAPH_CACHE = {}


def _get_graph(apply_affine: bool) -> bass.Bass:
    if apply_affine not in _GRAPH_CACHE:
        _GRAPH_CACHE[apply_affine] = _build_graph(apply_affine)
    return _GRAPH_CACHE[apply_affine]


def _prep_in_maps(x, Wq, bq, Wk, bk, Wv, bv, gamma, beta, apply_affine):
    scale = 1.0 / np.sqrt(np.float32(DK))
    in_maps = []
    for i in range(NCORES):
        dsl = slice(DC * i, DC * (i + 1))
        hsl = slice(HPC * i, HPC * (i + 1))
        x_sl = x[:, :, dsl]
        xt_full = x_sl.transpose(0, 2, 1).reshape(x.shape[0], HPC, DK, x.shape[1])
        xt_aug = np.concatenate(
            [xt_full, np.ones((x.shape[0], HPC, 1, x.shape[1]), np.float32)], axis=2
        )
        Wq_s = (Wq[hsl] * scale).astype(np.float64)
        bq_s = (bq[hsl] * scale).astype(np.float64)
        Wk_h = Wk[hsl].astype(np.float64)
        bk_h = bk[hsl].astype(np.float64)
        M = np.einsum("hde,hfe->hdf", Wk_h, Wq_s)      # [h, dK, dQ]
        u = np.einsum("hde,he->hd", Wk_h, bq_s)        # alpha coeffs (per t)
        wvec = np.einsum("hde,he->hd", Wq_s, bk_h)     # beta coeffs (per s)
        cconst = np.einsum("he,he->h", bk_h, bq_s)
        # lhsT for z: [d', dK | wvec]; z rows 0..63 = M@xh^T + u, row 64 = xh.w + c
        zw_np = np.concatenate(
            [M.transpose(0, 2, 1), wvec[:, :, None]], axis=2
        )  # [h, d'(=dQ... contraction dim), dK+1]
        zb_np = np.concatenate([u, cconst[:, None]], axis=1)[:, :, None]
        m = {
            "xt": np.ascontiguousarray(xt_aug).astype(BF_NP),
            "xs": np.ascontiguousarray(x_sl),
            "wpack": np.ascontiguousarray(
                np.concatenate(
                    [zw_np[0], zw_np[1], Wv[hsl][0], Wv[hsl][1]], axis=1
                )
            ).astype(BF_NP),
            "zb": np.ascontiguousarray(zb_np[:, :, 0].T).astype(np.float32),
            "bv16": np.ascontiguousarray(
                np.tile(bv[hsl][:, None, :], (1, 128, 16))
            ).astype(np.float32),
        }
        if apply_affine:
            m["gam"] = np.ascontiguousarray(
                np.tile(gamma[dsl][None, :], (128, 1))
            ).astype(np.float32)
            m["bet"] = np.ascontiguousarray(
                np.tile(beta[dsl][None, :], (128, 1))
            ).astype(np.float32)
        in_maps.append(m)
    return in_maps


def kernel(x, Wq, bq, Wk, bk, Wv, bv, gamma, beta):
    global LAST_RESULTS
    x = np.asarray(x, np.float32)
    Wq = np.asarray(Wq, np.float32)
    bq = np.asarray(bq, np.float32)
    Wk = np.asarray(Wk, np.float32)
    bk = np.asarray(bk, np.float32)
    Wv = np.asarray(Wv, np.float32)
    bv = np.asarray(bv, np.float32)
    gamma = np.asarray(gamma, np.float32)
    beta = np.asarray(beta, np.float32)

    apply_affine = not (
        np.allclose(gamma, 1.0, atol=0.0, rtol=0.0)
        and np.allclose(beta, 0.0, atol=0.0, rtol=0.0)
    )
    fake_ar = bool(int(os.environ.get("KERNEL_FAKE_AR", "0")))
    nc = _get_graph(apply_affine) if not fake_ar else _build_graph(apply_affine, fake_ar=True)

    in_maps = _prep_in_maps(x, Wq, bq, Wk, bk, Wv, bv, gamma, beta, apply_affine)

    res = run_bass_kernel_spmd(
        nc,
        in_maps,
        core_ids=list(range(NCORES)),
        trace=bool(int(os.environ.get("KERNEL_TRACE", "0"))),
    )
    LAST_RESULTS = res
    outs = [np.asarray(r["out"], np.float32) for r in res.results]
    return np.concatenate(outs, axis=2)


if __name__ == "__main__":
    nc = _build_graph(False)
    print("graph built ok:", len(nc.inst_map), "instructions")



# revision 50
# speedup vs baseline: 1.0586x; 1.0183x over previous
"""Trainium2 Bass kernel: per-head attention + residual + LayerNorm.

Problem shape: x [4, 2048, 1024], 16 heads of dk=64, causal softmax attention
with per-head Q/K/V linear projections, residual add, LayerNorm(D).

Sharding (8 cores): head-parallel. Core i owns heads (2i, 2i+1), i.e. feature
columns 128*i : 128*(i+1). Each core computes its feature slice of the output;
the only cross-core communication is a tiny per-batch AllReduce of LayerNorm
partial sums (sum and sum-of-squares over each core's 128 features). The host
shards/gathers and pre-transposes x (the [dk, S] layout each head needs).

Per-core kernel design (bf16 matmuls, fp32 PSUM accumulation):
- Scores via a host-precomputed bilinear form: scores^T = xh_aug^T @ z with
  z = [M @ xh^T + u | beta-row], M = Wk (Wq/sqrt(dk))^T. One projected tensor
  (z) instead of Q and K halves the PSUM->SBUF copies; x^T itself (with a
  built-in ones row for the bias terms) is the stationary matmul operand.
- Flash-style t-outer loop over 1024-col query halves; scores accumulate in
  PSUM [128,1024] chunks on a dedicated 3-deep ring (6 banks) that serves
  ONLY the score->exp pipeline; exp runs on ScalarE straight from PSUM in
  one instruction per (t-block, half), P is bf16 in SBUF. All other PSUM
  users (split O^T accumulators, projection staging, epilogue transposes)
  live on a separate ring of fast-release 1-bank [*,512] slots (2 banks),
  so no phase ever blocks the score pipeline's buffers.
- Causal mask: an identity-matmul accumulates -40 onto the upper triangle of
  the diagonal 128-block before exp (no vector-engine masking); sub-diagonal
  dead zones are simply never read by PV.
- PV accumulates O^T [65,1024] in PSUM with a ones-augmented V, so softmax
  denominators ride along as row 64; per 128-tile PE-transposes then let a
  single fused DVE op do (O*1/l + x) with the row-sum accumulated for free.
- rstd = exp(-0.5*ln(var+eps)) keeps every activation in one ACT table set
  (natural_log_exp_and_others; enforced by filtering the set map at compile).
- Emission is software-pipelined: the next pair's z/V projection is emitted
  mid-way through the current score loop, the transpose/normalize epilogue is
  deferred one unit, and per-half LayerNorm stats AllReduce in 8 small chunks
  so TensorE/ScalarE/VectorE and the collective overlap across units.

Self-contained: hardcodes all shapes; no sibling imports.
"""

import os
import numpy as np
import ml_dtypes

import concourse.bass as bass
import concourse.bacc as bacc
import concourse.mybir as mybir
from concourse.tile import TileContext
from concourse.bass_utils import run_bass_kernel_spmd

B, S, D, H = 4, 2048, 1024, 16
NCORES = 8
HPC = H // NCORES          # heads per core = 2
DK = D // H                # 64
DC = HPC * DK              # 128 feature cols per core
NT = S // 128              # 16 row tiles of 128
EPS = 1e-5
MASKNEG = -40.0
SPBUFS = 3
OPBUFS = 1
EPI_LAG = 1
HOOKJ = 6
QKBUFS = 3
PBUFS = 3
BF = mybir.dt.bfloat16
F32 = mybir.dt.float32
FP8 = mybir.dt.float8e4
FP8E5 = mybir.dt.float8e5
BF_NP = ml_dtypes.bfloat16
FP8_NP = ml_dtypes.float8_e4m3
FP8E5_NP = ml_dtypes.float8_e5m2
VW = 80  # padded V-tile row stride (65 used) so DoubleRow plane step is 16B-aligned
RG = [list(range(NCORES))]
A = mybir.AluOpType
AF = mybir.ActivationFunctionType

LAST_RESULTS = None  # BassKernelResults of the last run (for test harness)


def _build_graph(apply_affine: bool, B_: int = B, S_: int = S, rg=None, fake_ar: bool = False) -> bass.Bass:
    nc = bacc.Bacc()
    NT_ = S_ // 128
    if rg is None:
        rg = RG

    xt = nc.declare_dram_parameter("xt", [B_, HPC, DK + 1, S_], BF, isOutput=False)
    xs = nc.declare_dram_parameter("xs", [B_, S_, DC], F32, isOutput=False)
    wpack = nc.declare_dram_parameter(
        "wpack", [DK, HPC * (DK + 1) + HPC * DK], BF, isOutput=False
    )
    zb = nc.declare_dram_parameter("zb", [DK + 1, HPC], F32, isOutput=False)
    bv16 = nc.declare_dram_parameter("bv16", [HPC, 128, 16 * DK], F32, isOutput=False)
    if apply_affine:
        gam = nc.declare_dram_parameter("gam", [128, DC], F32, isOutput=False)
        bet = nc.declare_dram_parameter("bet", [128, DC], F32, isOutput=False)
    out = nc.declare_dram_parameter("out", [B_, S_, DC], F32, isOutput=True)

    # constants baked into the NEFF
    idn_h = nc.inline_tensor(np.eye(DK + 1, dtype=np.float32), name="idn")
    trineg_np = np.where(
        np.arange(128)[:, None] > np.arange(128)[None, :], MASKNEG, 0.0
    ).astype(np.float32)
    # DoubleRow-plane layout: [64, 2, 128] -> [64, 256], plane i = rows 64i..64i+63
    idn2_np = np.eye(128, dtype=np.float32).reshape(2, 64, 128).transpose(1, 0, 2)
    trineg2_np = trineg_np.reshape(2, 64, 128).transpose(1, 0, 2)
    imask_h = nc.inline_tensor(
        np.concatenate([idn2_np, trineg2_np], axis=2).reshape(64, 512).astype(
            FP8E5_NP
        ),
        name="imask",
    )

    # collective bounce buffers: LayerNorm stats per (batch, s-half):
    # [2(sum,sumsq), 128 rows, tiles-in-half]
    NHALF = (S_ + 1023) // 1024
    NTH = NT_ // NHALF
    stats_in = nc.dram_tensor("stats_in", [B_, NHALF, 2, 128, NTH], F32)
    stats_out = nc.dram_tensor(
        "stats_out", [B_, NHALF, 2, 128, NTH], F32, addr_space="Shared"
    )

    with TileContext(nc) as tc:
        with (
            tc.tile_pool(name="consts", bufs=1) as cpool,
            tc.tile_pool(name="sb", bufs=2) as sb,
            tc.tile_pool(name="ps", bufs=1, space="PSUM") as ps,
        ):
            # ---- load constants (single coalesced DMAs, first-needed first) ----
            wp_t = cpool.tile([DK, HPC * (DK + 1) + HPC * DK], BF, tag="wp")
            nc.sync.dma_start(out=wp_t[:], in_=wpack[:, :])
            zw_t = wp_t[:][:, 0 : HPC * (DK + 1)]
            wv_t = wp_t[:][:, HPC * (DK + 1) : HPC * (DK + 1) + HPC * DK]
            # queue order tuned for the first exp: wpack, then the first
            # half of head-0 x^T (all the first projection needs), then the
            # small constants the first z-copy and diagonal mask need, then
            # the rest of batch-0 x^T
            xth0 = [
                sb.tile([DK + 1, S_], BF, tag="xth", name=f"xth0_{h2}", bufs=B_ * HPC)
                for h2 in range(HPC)
            ]
            nc.sync.dma_start(out=xth0[0][:, 0 : S_ // 2], in_=xt[0, 0, :, 0 : S_ // 2])
            zbq_t = cpool.tile([DK + 1, HPC], F32, tag="zb")
            nc.sync.dma_start(out=zbq_t[:], in_=zb[:, :])
            imaskq_t = cpool.tile([64, 512], FP8E5, tag="imask")
            nc.sync.dma_start(out=imaskq_t[:], in_=imask_h[:, :])
            nc.sync.dma_start(out=xth0[0][:, S_ // 2 : S_], in_=xt[0, 0, :, S_ // 2 : S_])
            # head-1 x^T on the gpsimd queue: overlaps head-0's sync-queue
            # transfers, and nothing before the h0 scores needs it
            nc.gpsimd.dma_start(out=xth0[1][:, 0 : S_ // 2], in_=xt[0, 1, :, 0 : S_ // 2])
            nc.gpsimd.dma_start(out=xth0[1][:, S_ // 2 : S_], in_=xt[0, 1, :, S_ // 2 : S_])
            idn_t = cpool.tile([DK + 1, DK + 1], F32, tag="idn")
            nc.gpsimd.dma_start(out=idn_t[:], in_=idn_h[:, :])
            bv16_t = cpool.tile([128, HPC * 16 * DK], F32, tag="bv16")
            for h in range(HPC):
                nc.gpsimd.dma_start(
                    out=bv16_t[:, 16 * DK * h : 16 * DK * (h + 1)], in_=bv16[h]
                )
            if apply_affine:
                gam_t = cpool.tile([128, DC], F32, tag="gam")
                nc.sync.dma_start(out=gam_t[:], in_=gam[:, :])
                bet_t = cpool.tile([128, DC], F32, tag="bet")
                nc.sync.dma_start(out=bet_t[:], in_=bet[:, :])

            zb_t = zbq_t
            imask3 = imaskq_t[:].rearrange("p (i c) -> p i c", c=256)
            idn128_t = imask3[:, :, 0:128]
            maskt_t = imask3[:, :, 128:256]
            eps_t = cpool.tile([128, 1], F32, tag="eps")
            nc.vector.memset(eps_t[:], EPS)

            def _emit_stats(b, y_b, acc, ch):
                # LayerNorm partial stats + AllReduce for one s-half.
                # One fused [sum|sumsq] tile -> one DMA.
                t0, t1 = ch * NTH, ch * NTH + NTH
                stq = sb.tile([128, 2 * NTH], F32, tag="sums", bufs=3)
                nc.vector.tensor_add(
                    stq[:, 0:NTH], acc[0][:, t0:t1], acc[1][:, t0:t1]
                )
                for i in range(t0, t1):
                    scr = sb.tile([128, 128], F32, tag="scr")
                    nc.vector.scalar_tensor_tensor(
                        scr[:],
                        y_b[:, 128 * i : 128 * i + 128],
                        1.0,
                        y_b[:, 128 * i : 128 * i + 128],
                        A.mult,
                        A.mult,
                        accum_out=stq[:, NTH + i - t0 : NTH + i - t0 + 1],
                    )
                nc.sync.dma_start(
                    out=stats_in[b, ch].rearrange("c p t -> p c t"),
                    in_=stq[:].rearrange("p (c t) -> p c t", t=NTH),
                )
                if fake_ar:
                    nc.sync.dma_start(out=stats_out[b, ch], in_=stats_in[b, ch])
                else:
                    nc.gpsimd.collective_compute(
                        "AllReduce",
                        A.add,
                        replica_groups=rg,
                        ins=[stats_in[b, ch].opt()],
                        outs=[stats_out[b, ch].opt()],
                    )

            def emit_ln(b, ch, y_b, split=False):
                t0 = ch * NTH
                red = sb.tile([128, 2 * NTH], F32, tag="red", bufs=3)
                nc.sync.dma_start(
                    out=red[:].rearrange("p (c t) -> p c t", t=NTH),
                    in_=stats_out[b, ch].rearrange("c p t -> p c t"),
                )
                mean = sb.tile([128, NTH], F32, tag="mean", bufs=3)
                nc.vector.tensor_scalar(
                    mean[:], red[:, 0:NTH], 1.0 / D, None, A.mult
                )
                msq = sb.tile([128, NTH], F32, tag="msq", bufs=3)
                nc.vector.tensor_mul(msq[:], mean[:], mean[:])
                var = sb.tile([128, NTH], F32, tag="var", bufs=3)
                nc.vector.scalar_tensor_tensor(
                    var[:], red[:, NTH : 2 * NTH], 1.0 / D, msq[:], A.mult,
                    A.subtract,
                )
                lnv = sb.tile([128, NTH], F32, tag="lnv", bufs=3)
                nc.scalar.activation(lnv[:], var[:], AF.Ln, bias=eps_t[:])
                rstd = sb.tile([128, NTH], F32, tag="rstd", bufs=3)
                nc.scalar.activation(rstd[:], lnv[:], AF.Exp, scale=-0.5)
                ostb = sb.tile([128, 128 * NTH], F32, tag="ost", bufs=2)
                eng = nc.gpsimd if ((b * NHALF + ch) % 2 == 0 and b < B_ - 1) else nc.sync
                hn = NTH // 2
                for k in range(NTH):
                    i = t0 + k
                    nc.vector.tensor_scalar(
                        ostb[:, 128 * k : 128 * k + 128],
                        y_b[:, 128 * i : 128 * i + 128],
                        mean[:, k : k + 1],
                        rstd[:, k : k + 1],
                        A.subtract,
                        A.mult,
                    )
                    if apply_affine:
                        nc.vector.tensor_mul(
                            ostb[:, 128 * k : 128 * k + 128],
                            ostb[:, 128 * k : 128 * k + 128],
                            gam_t[:],
                        )
                        nc.vector.tensor_add(
                            ostb[:, 128 * k : 128 * k + 128],
                            ostb[:, 128 * k : 128 * k + 128],
                            bet_t[:],
                        )
                    if split and k == hn - 1:
                        # first-half store overlaps the remaining normalizes
                        eng.dma_start(
                            out=out[b, 128 * t0 : 128 * (t0 + hn), :].rearrange(
                                "(i p) d -> p i d", p=128
                            ),
                            in_=ostb[:, 0 : 128 * hn].rearrange(
                                "p (i d) -> p i d", d=128
                            ),
                        )
                lo_t = t0 + hn if split else t0
                eng.dma_start(
                    out=out[b, 128 * lo_t : 128 * (t0 + NTH), :].rearrange(
                        "(i p) d -> p i d", p=128
                    ),
                    in_=ostb[:, 128 * (lo_t - t0) :].rearrange(
                        "p (i d) -> p i d", d=128
                    ),
                )

            y_tiles = {}
            bstate = {}
            pstate = {}
            pw = min(1024, S_)
            NP = B_ * HPC

            def emit_proj(pair):
                b, hh = divmod(pair, HPC)
                if hh == 0:
                    if b == 0:
                        xth = xth0
                    else:
                        xth = [None, None]
                        for h2 in range(HPC):
                            xth[h2] = sb.tile(
                                [DK + 1, S_], BF, tag="xth", name=f"xth{b}_{h2}", bufs=B_ * HPC
                            )
                            eng = nc.sync if h2 == 0 else nc.gpsimd
                            eng.dma_start(
                                out=xth[h2][:, 0 : S_ // 2], in_=xt[b, h2, :, 0 : S_ // 2]
                            )
                            eng.dma_start(
                                out=xth[h2][:, S_ // 2 : S_], in_=xt[b, h2, :, S_ // 2 : S_]
                            )
                    xs_b = sb.tile([128, S_], F32, tag="xs", name=f"xs{b}")
                    y_b = sb.tile([128, S_], F32, tag=f"y{b}", name=f"y{b}")
                    y_tiles[b] = y_b
                    bstate[b] = (xth, xs_b, y_b, {})
                    need_xs_dma = True
                else:
                    need_xs_dma = False
                xth, xs_b, y_b, accs = bstate[b]
                xh = xth[hh]
                # z = [M @ xh^T + u | beta-row]: scores become xh_aug^T @ z.
                # Emit per s-half (z chunks then V group) so the first scores
                # only wait on the first half of the x^T DMA.
                z = sb.tile([DK + 1, S_], BF, tag="z", name=f"z{pair}", bufs=NP)
                v = sb.tile([128, NT_ * VW], FP8, tag="v", name=f"v{pair}", bufs=NP)
                v3 = v[:].rearrange("p (t w) -> p t w", w=VW)
                nc.vector.memset(v3[:, :, DK : DK + 1], 1.0)
                gv = min(8, NT_)
                for half in range(NT_ // gv):
                    for c in (2 * half, 2 * half + 1):
                        zp = ps.tile([128, 512], F32, tag="op", bufs=2, name=f"zp{c}")
                        nc.tensor.matmul(
                            zp[0 : DK + 1, :],
                            lhsT=zw_t[:, (DK + 1) * hh : (DK + 1) * (hh + 1)],
                            rhs=xh[0:DK, 512 * c : 512 * c + 512],
                            start=True,
                            stop=True,
                        )
                        nc.vector.tensor_scalar(
                            z[:, 512 * c : 512 * c + 512],
                            zp[0 : DK + 1, :],
                            zb_t[:, hh : hh + 1],
                            None,
                            A.add,
                        )
                    g = half
                    vp = ps.tile([128, 512], F32, tag="op", bufs=2, name=f"vp{g}")
                    for u in range(gv):
                        j = gv * g + u
                        nc.tensor.matmul(
                            vp[:, DK * u : DK * u + DK],
                            lhsT=xh[0:DK, 128 * j : 128 * j + 128],
                            rhs=wv_t[:, hh * DK : hh * DK + DK],
                            start=True,
                            stop=True,
                        )
                    nc.vector.tensor_tensor(
                        v3[:, gv * g : gv * g + gv, 0:DK],
                        vp[:, 0 : gv * DK].rearrange("q (t w) -> q t w", w=DK),
                        bv16_t[:].rearrange("q (h t w) -> q (h t) w", h=HPC, w=DK)[
                            :, hh * 16 : hh * 16 + gv, :
                        ],
                        A.add,
                    )
                if need_xs_dma:
                    nc.sync.dma_start(
                        out=xs_b[:].rearrange("p (i d) -> p i d", d=128),
                        in_=xs[b].rearrange("(i p) d -> p i d", p=128),
                    )
                acc_h = sb.tile([128, NT_], F32, tag=f"acc{hh}", name=f"acc{pair}", bufs=B_)
                accs[hh] = acc_h
                pstate[pair] = (xh, z, v3, acc_h)

            def emit_jhalf(pair, hs, mid_hook=None, service_hook=None, final=False):
                """Score/exp/PV loop for one 1024-col s-half; returns the
                deferred transpose/normalize epilogue closure. PV runs as
                fp8 DoubleRow over PAIRS of key-tiles (j, j+1): V planes are
                adjacent slices of v3, P planes are halves of a shared fp8
                tile — one matmul covers two key-tiles at 0.5 cycles/col."""
                b, hh = divmod(pair, HPC)
                xh, z, v3, acc_h = pstate[pair]
                _, xs_b, y_b, accs = bstate[b]
                he = min(S_, hs + 1024)
                w = he - hs
                nj = he // 128
                npair_t = nj // 2
                opA = ps.tile([DK + 1, 512], F32, tag="op", bufs=2)
                opB = ps.tile([DK + 1, 512], F32, tag="op", bufs=2)
                # last tile-pair touching each 512-col bank (for stop flags)
                last_m = [
                    max(
                        mm
                        for mm in range(npair_t)
                        if max(0, 256 * mm - hs) < 512 * (g + 1)
                    )
                    for g in range(w // 512)
                ]
                ot = sb.tile([DK + 1, 1024], F32, tag="ot", bufs=2 + EPI_LAG)

                def _epi_piece(g, stq):
                    """Inline transpose/normalize + stats fill for one 512-col
                    bank of the FINAL half (tail-latency special case)."""
                    t0 = hs // 128 + 4 * g
                    tp = ps.tile([128, 512], F32, tag="op", bufs=2, name=f"ftp{g}")
                    for k in range(4):
                        nc.tensor.transpose(
                            tp[:, 128 * k : 128 * k + DK + 1],
                            ot[:, 512 * g + 128 * k : 512 * g + 128 * k + 128],
                            idn_t[:],
                        )
                    r4 = sb.tile([128, 4], F32, tag="r8", bufs=3, name=f"fr4{g}")
                    nc.vector.reciprocal(
                        r4[:],
                        tp[:].rearrange("q (k c) -> q k c", c=128)[:, 0:4, DK : DK + 1],
                    )
                    for k in range(4):
                        i = t0 + k
                        nc.vector.scalar_tensor_tensor(
                            y_b[:, 128 * i + DK * hh : 128 * i + DK * hh + DK],
                            tp[:, 128 * k : 128 * k + DK],
                            r4[:, k : k + 1],
                            xs_b[:, 128 * i + DK * hh : 128 * i + DK * hh + DK],
                            A.mult,
                            A.add,
                            accum_out=acc_h[:, i : i + 1],
                        )
                    nc.vector.tensor_add(
                        stq[:, 4 * g : 4 * g + 4],
                        accs[0][:, t0 : t0 + 4],
                        acc_h[:, t0 : t0 + 4],
                    )
                    for k in range(4):
                        i = t0 + k
                        scr = sb.tile([128, 128], F32, tag="scr")
                        nc.vector.scalar_tensor_tensor(
                            scr[:],
                            y_b[:, 128 * i : 128 * i + 128],
                            1.0,
                            y_b[:, 128 * i : 128 * i + 128],
                            A.mult,
                            A.mult,
                            accum_out=stq[:, NTH + 4 * g + k : NTH + 4 * g + k + 1],
                        )

                stq_f = None
                if final:
                    stq_f = sb.tile([128, 2 * NTH], F32, tag="sums", bufs=3, name="stqf")

                prev_pv = None
                for m in range(npair_t):
                    p2m = sb.tile(
                        [128, 2048], FP8, tag="p", bufs=PBUFS, name=f"p2_{m}"
                    )
                    p3 = p2m[:].rearrange("q (i c) -> q i c", c=1024)
                    los = [0, 0]
                    for par in range(2):
                        j = 2 * m + par
                        s0 = 128 * j
                        rel = s0 - hs
                        sp = ps.tile([128, 1024], F32, tag="sp", bufs=SPBUFS)
                        if rel < 0:
                            ss = 0
                            while ss < w:
                                sl = min(512, w - ss)
                                nc.tensor.matmul(
                                    sp[:, ss : ss + sl],
                                    lhsT=xh[:, s0 : s0 + 128],
                                    rhs=z[:, hs + ss : hs + ss + sl],
                                    start=True,
                                    stop=True,
                                )
                                ss += sl
                            lo = 0
                        else:
                            lo = rel
                            nc.tensor.matmul(
                                sp[:, rel : rel + 128],
                                lhsT=idn128_t,
                                rhs=maskt_t,
                                start=True,
                                stop=False,
                                perf_mode=mybir.MatmulPerfMode.DoubleRow,
                                skip_group_check=True,
                            )
                            nc.tensor.matmul(
                                sp[:, rel : rel + 128],
                                lhsT=xh[:, s0 : s0 + 128],
                                rhs=z[:, s0 : s0 + 128],
                                start=False,
                                stop=True,
                                skip_group_check=True,
                            )
                            ss = rel + 128
                            while ss < w:
                                sl = min(512 - (ss % 512), w - ss)
                                nc.tensor.matmul(
                                    sp[:, ss : ss + sl],
                                    lhsT=xh[:, s0 : s0 + 128],
                                    rhs=z[:, hs + ss : hs + ss + sl],
                                    start=True,
                                    stop=True,
                                )
                                ss += sl
                        los[par] = lo
                        nc.scalar.activation(p3[:, par, lo:w], sp[:, lo:w], AF.Exp)
                        if j == min(HOOKJ, nj - 1) and mid_hook is not None:
                            mid_hook()
                    if m in (1, 4) and service_hook is not None:
                        service_hook()

                    # PV for the pair, deferred by one pair so the PE computes
                    # the next scores while ACT exps this pair
                    def _pv(m=m, p3=p3, lo0=los[0], lo1=los[1]):
                        j0 = 2 * m
                        # ragged diagonal: [lo0, lo1) has only plane 0 valid
                        cs = lo0
                        while cs < lo1:
                            ce = min(512 * (cs // 512) + 512, lo1)
                            opt = opA if cs < 512 else opB
                            nc.tensor.matmul(
                                opt[:, cs % 512 : cs % 512 + (ce - cs)],
                                lhsT=v3[:, j0, 0 : DK + 1],
                                rhs=p3[:, 0, cs:ce],
                                start=(m == 0),
                                stop=False,
                                skip_group_check=True,
                            )
                            cs = ce
                        # both planes valid: DoubleRow over (j0, j0+1)
                        cs = lo1
                        while cs < w:
                            ce = min(512 * (cs // 512) + 512, w)
                            g = cs // 512
                            opt = opA if cs < 512 else opB
                            first = (m == 0) and (cs == lo1 == lo0 or cs >= 512)
                            nc.tensor.matmul(
                                opt[:, cs % 512 : cs % 512 + (ce - cs)],
                                lhsT=v3[:, j0 : j0 + 2, 0 : DK + 1],
                                rhs=p3[:, :, cs:ce],
                                start=first,
                                stop=(m == last_m[g] and ce == min(w, 512 * (g + 1))),
                                perf_mode=mybir.MatmulPerfMode.DoubleRow,
                                skip_group_check=True,
                            )
                            cs = ce

                    if prev_pv is not None:
                        prev_pv()
                        if final and m - 1 == last_m[0]:
                            # bank 0 fully accumulated: drain + normalize it
                            # while the tail PVs of bank 1 still run
                            nc.vector.tensor_copy(ot[:, 0:512], opA[:, 0:512])
                            _epi_piece(0, stq_f)
                    prev_pv = _pv
                if prev_pv is not None:
                    prev_pv()
                if not final:
                    nc.vector.tensor_copy(ot[:, 0:512], opA[:, 0:512])
                nc.vector.tensor_copy(ot[:, 512:w], opB[:, 0 : w - 512])
                if final:
                    _epi_piece(1, stq_f)
                    nc.sync.dma_start(
                        out=stats_in[b, 1].rearrange("c p t -> p c t"),
                        in_=stq_f[:].rearrange("p (c t) -> p c t", t=NTH),
                    )
                    if fake_ar:
                        nc.sync.dma_start(out=stats_out[b, 1], in_=stats_in[b, 1])
                    else:
                        nc.gpsimd.collective_compute(
                            "AllReduce",
                            A.add,
                            replica_groups=rg,
                            ins=[stats_in[b, 1].opt()],
                            outs=[stats_out[b, 1].opt()],
                        )
                    emit_ln(b, 1, y_b)
                    return None

                def _epilogue():
                    nk = he // 128 - hs // 128
                    tps = [
                        ps.tile([128, 512], F32, tag="op", bufs=2, name=f"tp{g}")
                        for g in range((nk + 3) // 4)
                    ]
                    for i in range(hs // 128, he // 128):
                        k = i - hs // 128
                        tp = tps[k // 4]
                        nc.tensor.transpose(
                            tp[:, 128 * (k % 4) : 128 * (k % 4) + DK + 1],
                            ot[:, 128 * i - hs : 128 * i - hs + 128],
                            idn_t[:],
                        )
                    r8 = sb.tile([128, 8], F32, tag="r8", bufs=3)
                    for g, tp in enumerate(tps):
                        gn = min(4, nk - 4 * g)
                        nc.vector.reciprocal(
                            r8[:, 4 * g : 4 * g + gn],
                            tp[:].rearrange("q (k c) -> q k c", c=128)[
                                :, 0:gn, DK : DK + 1
                            ],
                        )
                    for i in range(hs // 128, he // 128):
                        k = i - hs // 128
                        tp = tps[k // 4]
                        nc.vector.scalar_tensor_tensor(
                            y_b[:, 128 * i + DK * hh : 128 * i + DK * hh + DK],
                            tp[:, 128 * (k % 4) : 128 * (k % 4) + DK],
                            r8[:, k : k + 1],
                            xs_b[:, 128 * i + DK * hh : 128 * i + DK * hh + DK],
                            A.mult,
                            A.add,
                            accum_out=acc_h[:, i : i + 1],
                        )
                    if hh == HPC - 1:
                        _emit_stats(b, y_b, accs, hs // 1024)
                        ln_ready.append((b, hs // 1024))

                return _epilogue

            emit_proj(0)
            pending = []
            ln_ready = []
            for pair in range(NP):
                for k, hs in enumerate(range(0, S_, 1024)):
                    hook = None
                    if k == 0 and pair + 1 < NP:
                        hook = (lambda pr=pair: emit_proj(pr + 1))
                    svc = None
                    if pair == NP - 1 and k == 1:
                        # pop the last pair's h0 epilogue mid-h1 so its LN
                        # stats chain overlaps the remaining exp tiles
                        svc = lambda: pending.pop(0)() if pending else None
                    epi = emit_jhalf(pair, hs, mid_hook=hook, service_hook=svc)
                    if epi is not None:
                        pending.append(epi)
                    if len(pending) > EPI_LAG:
                        pending.pop(0)()
                    if pair >= NP - 2:
                        # drain earlier batches' LN during the last two pairs
                        # so only the final batch's LN remains in the tail
                        for _ in range(2):
                            if ln_ready:
                                bb, cc = ln_ready.pop(0)
                                emit_ln(bb, cc, y_tiles[bb])
            for e in pending:
                e()
            while ln_ready:
                bb, cc = ln_ready.pop(0)
                emit_ln(bb, cc, y_tiles[bb], split=not ln_ready)


    # Restrict Exp/Ln to the shared natural_log_exp_and_others table set so
    # the whole kernel uses one ACT table load (indices preserved).
    import concourse.bacc as _bacc_mod

    _orig_tables = _bacc_mod.get_activation_tables

    def _filtered_tables(arch):
        out = {}
        for name, fns in _orig_tables(arch).items():
            if name != "natural_log_exp_and_others":
                fns = set(fns) - {AF.Exp, AF.Ln}
            out[name] = fns
        return out

    _bacc_mod.get_activation_tables = _filtered_tables
    try:
        nc.compile()
    finally:
        _bacc_mod.get_activation_tables = _orig_tables
    return nc


_GRAPH_CACHE = {}


def _get_graph(apply_affine: bool) -> bass.Bass:
    if apply_affine not in _GRAPH_CACHE:
        _GRAPH_CACHE[apply_affine] = _build_graph(apply_affine)
    return _GRAPH_CACHE[apply_affine]


def _prep_in_maps(x, Wq, bq, Wk, bk, Wv, bv, gamma, beta, apply_affine):
    scale = 1.0 / np.sqrt(np.float32(DK))
    in_maps = []
    for i in range(NCORES):
        dsl = slice(DC * i, DC * (i + 1))
        hsl = slice(HPC * i, HPC * (i + 1))
        x_sl = x[:, :, dsl]
        xt_full = x_sl.transpose(0, 2, 1).reshape(x.shape[0], HPC, DK, x.shape[1])
        xt_aug = np.concatenate(
            [xt_full, np.ones((x.shape[0], HPC, 1, x.shape[1]), np.float32)], axis=2
        )
        Wq_s = (Wq[hsl] * scale).astype(np.float64)
        bq_s = (bq[hsl] * scale).astype(np.float64)
        Wk_h = Wk[hsl].astype(np.float64)
        bk_h = bk[hsl].astype(np.float64)
        M = np.einsum("hde,hfe->hdf", Wk_h, Wq_s)      # [h, dK, dQ]
        u = np.einsum("hde,he->hd", Wk_h, bq_s)        # alpha coeffs (per t)
        wvec = np.einsum("hde,he->hd", Wq_s, bk_h)     # beta coeffs (per s)
        cconst = np.einsum("he,he->h", bk_h, bq_s)
        # lhsT for z: [d', dK | wvec]; z rows 0..63 = M@xh^T + u, row 64 = xh.w + c
        zw_np = np.concatenate(
            [M.transpose(0, 2, 1), wvec[:, :, None]], axis=2
        )  # [h, d'(=dQ... contraction dim), dK+1]
        zb_np = np.concatenate([u, cconst[:, None]], axis=1)[:, :, None]
        m = {
            "xt": np.ascontiguousarray(xt_aug).astype(BF_NP),
            "xs": np.ascontiguousarray(x_sl),
            "wpack": np.ascontiguousarray(
                np.concatenate(
                    [zw_np[0], zw_np[1], Wv[hsl][0], Wv[hsl][1]], axis=1
                )
            ).astype(BF_NP),
            "zb": np.ascontiguousarray(zb_np[:, :, 0].T).astype(np.float32),
            "bv16": np.ascontiguousarray(
                np.tile(bv[hsl][:, None, :], (1, 128, 16))
            ).astype(np.float32),
        }
        if apply_affine:
            m["gam"] = np.ascontiguousarray(
                np.tile(gamma[dsl][None, :], (128, 1))
            ).astype(np.float32)
            m["bet"] = np.ascontiguousarray(
                np.tile(beta[dsl][None, :], (128, 1))
            ).astype(np.float32)
        in_maps.append(m)
    return in_maps


def kernel(x, Wq, bq, Wk, bk, Wv, bv, gamma, beta):
    global LAST_RESULTS
    x = np.asarray(x, np.float32)
    Wq = np.asarray(Wq, np.float32)
    bq = np.asarray(bq, np.float32)
    Wk = np.asarray(Wk, np.float32)
    bk = np.asarray(bk, np.float32)
    Wv = np.asarray(Wv, np.float32)
    bv = np.asarray(bv, np.float32)
    gamma = np.asarray(gamma, np.float32)
    beta = np.asarray(beta, np.float32)

    apply_affine = not (
        np.allclose(gamma, 1.0, atol=0.0, rtol=0.0)
        and np.allclose(beta, 0.0, atol=0.0, rtol=0.0)
    )
    fake_ar = bool(int(os.environ.get("KERNEL_FAKE_AR", "0")))
    nc = _get_graph(apply_affine) if not fake_ar else _build_graph(apply_affine, fake_ar=True)

    in_maps = _prep_in_maps(x, Wq, bq, Wk, bk, Wv, bv, gamma, beta, apply_affine)

    res = run_bass_kernel_spmd(
        nc,
        in_maps,
        core_ids=list(range(NCORES)),
        trace=bool(int(os.environ.get("KERNEL_TRACE", "0"))),
    )
    LAST_RESULTS = res
    outs = [np.asarray(r["out"], np.float32) for r in res.results]
    return np.concatenate(outs, axis=2)


if __name__ == "__main__":
    nc = _build_graph(False)
    print("graph built ok:", len(nc.inst_map), "instructions")



# revision 75
# speedup vs baseline: 1.0885x; 1.0283x over previous
"""Trainium2 Bass kernel: per-head attention + residual + LayerNorm.

Problem shape: x [4, 2048, 1024], 16 heads of dk=64, causal softmax attention
with per-head Q/K/V linear projections, residual add, LayerNorm(D).

Sharding (8 cores): head-parallel. Core i owns heads (2i, 2i+1), i.e. feature
columns 128*i : 128*(i+1). Each core computes its feature slice of the output;
the only cross-core communication is a tiny per-batch AllReduce of LayerNorm
partial sums (sum and sum-of-squares over each core's 128 features). The host
shards/gathers and pre-transposes x (the [dk, S] layout each head needs).

Per-core kernel design (bf16 matmuls, fp32 PSUM accumulation):
- Scores via a host-precomputed bilinear form: scores^T = xh_aug^T @ z with
  z = [M @ xh^T + u | beta-row], M = Wk (Wq/sqrt(dk))^T. One projected tensor
  (z) instead of Q and K halves the PSUM->SBUF copies; x^T itself (with a
  built-in ones row for the bias terms) is the stationary matmul operand.
- Flash-style t-outer loop over 1024-col query halves; scores accumulate in
  PSUM [128,1024] chunks on a dedicated 3-deep ring (6 banks) that serves
  ONLY the score->exp pipeline; exp runs on ScalarE straight from PSUM in
  one instruction per (t-block, half), P is bf16 in SBUF. All other PSUM
  users (split O^T accumulators, projection staging, epilogue transposes)
  live on a separate ring of fast-release 1-bank [*,512] slots (2 banks),
  so no phase ever blocks the score pipeline's buffers.
- Causal mask: an identity-matmul accumulates -40 onto the upper triangle of
  the diagonal 128-block before exp (no vector-engine masking); sub-diagonal
  dead zones are simply never read by PV.
- PV accumulates O^T [65,1024] in PSUM with a ones-augmented V, so softmax
  denominators ride along as row 64; per 128-tile PE-transposes then let a
  single fused DVE op do (O*1/l + x) with the row-sum accumulated for free.
- rstd = exp(-0.5*ln(var+eps)) keeps every activation in one ACT table set
  (natural_log_exp_and_others; enforced by filtering the set map at compile).
- Emission is software-pipelined: the next pair's z/V projection is emitted
  mid-way through the current score loop, the transpose/normalize epilogue is
  deferred one unit, and per-half LayerNorm stats AllReduce in 8 small chunks
  so TensorE/ScalarE/VectorE and the collective overlap across units.

Self-contained: hardcodes all shapes; no sibling imports.
"""

import os
import numpy as np
import ml_dtypes

import concourse.bass as bass
import concourse.bacc as bacc
import concourse.mybir as mybir
from concourse.tile import TileContext
from concourse.bass_utils import run_bass_kernel_spmd

B, S, D, H = 4, 2048, 1024, 16
NCORES = 8
HPC = H // NCORES          # heads per core = 2
DK = D // H                # 64
DC = HPC * DK              # 128 feature cols per core
NT = S // 128              # 16 row tiles of 128
EPS = 1e-5
MASKNEG = -40.0
SPBUFS = 3
OPBUFS = 1
EPI_LAG = 1
HOOKJ = 6
QKBUFS = 3
PBUFS = 6
BF = mybir.dt.bfloat16
F32 = mybir.dt.float32
FP8 = mybir.dt.float8e4
FP8E5 = mybir.dt.float8e5
BF_NP = ml_dtypes.bfloat16
FP8_NP = ml_dtypes.float8_e4m3
FP8E5_NP = ml_dtypes.float8_e5m2
VW = 80  # padded V-tile row stride (65 used) so DoubleRow plane step is 16B-aligned
RG = [list(range(NCORES))]
A = mybir.AluOpType
AF = mybir.ActivationFunctionType

LAST_RESULTS = None  # BassKernelResults of the last run (for test harness)


def _build_graph(apply_affine: bool, B_: int = B, S_: int = S, rg=None, fake_ar: bool = False) -> bass.Bass:
    nc = bacc.Bacc()
    NT_ = S_ // 128
    if rg is None:
        rg = RG

    xt = nc.declare_dram_parameter("xt", [B_, HPC, DK + 1, S_], BF, isOutput=False)
    xs = nc.declare_dram_parameter("xs", [B_, S_, DC], F32, isOutput=False)
    wpack = nc.declare_dram_parameter(
        "wpack", [DK, HPC * (DK + 1) + HPC * DK], BF, isOutput=False
    )
    zb = nc.declare_dram_parameter("zb", [DK + 1, HPC], F32, isOutput=False)
    bv16 = nc.declare_dram_parameter("bv16", [HPC, 128, 16 * DK], F32, isOutput=False)
    if apply_affine:
        gam = nc.declare_dram_parameter("gam", [128, DC], F32, isOutput=False)
        bet = nc.declare_dram_parameter("bet", [128, DC], F32, isOutput=False)
    out = nc.declare_dram_parameter("out", [B_, S_, DC], F32, isOutput=True)

    # constants baked into the NEFF
    idn_h = nc.inline_tensor(np.eye(DK + 1, dtype=np.float32), name="idn")
    trineg_np = np.where(
        np.arange(128)[:, None] > np.arange(128)[None, :], MASKNEG, 0.0
    ).astype(np.float32)
    # DoubleRow-plane layout: [64, 2, 128] -> [64, 256], plane i = rows 64i..64i+63
    idn2_np = np.eye(128, dtype=np.float32).reshape(2, 64, 128).transpose(1, 0, 2)
    trineg2_np = trineg_np.reshape(2, 64, 128).transpose(1, 0, 2)
    imask_h = nc.inline_tensor(
        np.concatenate([idn2_np, trineg2_np], axis=2).reshape(64, 512).astype(
            FP8E5_NP
        ),
        name="imask",
    )

    # collective bounce buffers: LayerNorm stats per (batch, s-half):
    # [2(sum,sumsq), 128 rows, tiles-in-half]
    NHALF = (S_ + 1023) // 1024
    NTH = NT_ // NHALF
    stats_in = nc.dram_tensor("stats_in", [B_, NHALF, 2, 128, NTH], F32)
    stats_out = nc.dram_tensor(
        "stats_out", [B_, NHALF, 2, 128, NTH], F32, addr_space="Shared"
    )

    with TileContext(nc) as tc:
        with (
            tc.tile_pool(name="consts", bufs=1) as cpool,
            tc.tile_pool(name="sb", bufs=2) as sb,
            tc.tile_pool(name="ps", bufs=1, space="PSUM") as ps,
        ):
            # ---- load constants (single coalesced DMAs, first-needed first) ----
            wp_t = cpool.tile([DK, HPC * (DK + 1) + HPC * DK], BF, tag="wp")
            nc.sync.dma_start(out=wp_t[:], in_=wpack[:, :])
            zw_t = wp_t[:][:, 0 : HPC * (DK + 1)]
            wv_t = wp_t[:][:, HPC * (DK + 1) : HPC * (DK + 1) + HPC * DK]
            # queue order tuned for the first exp: wpack, then the first
            # half of head-0 x^T (all the first projection needs), then the
            # small constants the first z-copy and diagonal mask need, then
            # the rest of batch-0 x^T
            xth0 = [
                sb.tile([DK + 1, S_], BF, tag="xth", name=f"xth0_{h2}", bufs=B_ * HPC)
                for h2 in range(HPC)
            ]
            nc.sync.dma_start(out=xth0[0][:, 0 : S_ // 2], in_=xt[0, 0, :, 0 : S_ // 2])
            zbq_t = cpool.tile([DK + 1, HPC], F32, tag="zb")
            nc.sync.dma_start(out=zbq_t[:], in_=zb[:, :])
            imaskq_t = cpool.tile([64, 512], FP8E5, tag="imask")
            nc.sync.dma_start(out=imaskq_t[:], in_=imask_h[:, :])
            nc.sync.dma_start(out=xth0[0][:, S_ // 2 : S_], in_=xt[0, 0, :, S_ // 2 : S_])
            # head-1 x^T on the gpsimd queue: overlaps head-0's sync-queue
            # transfers, and nothing before the h0 scores needs it
            nc.gpsimd.dma_start(out=xth0[1][:, 0 : S_ // 2], in_=xt[0, 1, :, 0 : S_ // 2])
            nc.gpsimd.dma_start(out=xth0[1][:, S_ // 2 : S_], in_=xt[0, 1, :, S_ // 2 : S_])
            idn_t = cpool.tile([DK + 1, DK + 1], F32, tag="idn")
            nc.gpsimd.dma_start(out=idn_t[:], in_=idn_h[:, :])
            bv16_t = cpool.tile([128, HPC * 16 * DK], F32, tag="bv16")
            for h in range(HPC):
                nc.gpsimd.dma_start(
                    out=bv16_t[:, 16 * DK * h : 16 * DK * (h + 1)], in_=bv16[h]
                )
            if apply_affine:
                gam_t = cpool.tile([128, DC], F32, tag="gam")
                nc.sync.dma_start(out=gam_t[:], in_=gam[:, :])
                bet_t = cpool.tile([128, DC], F32, tag="bet")
                nc.sync.dma_start(out=bet_t[:], in_=bet[:, :])

            zb_t = zbq_t
            imask3 = imaskq_t[:].rearrange("p (i c) -> p i c", c=256)
            idn128_t = imask3[:, :, 0:128]
            maskt_t = imask3[:, :, 128:256]
            eps_t = cpool.tile([128, 1], F32, tag="eps")
            nc.vector.memset(eps_t[:], EPS)

            def _emit_stats(b, y_b, acc, ch):
                # LayerNorm partial stats + AllReduce for one s-half.
                # One fused [sum|sumsq] tile -> one DMA.
                t0, t1 = ch * NTH, ch * NTH + NTH
                stq = sb.tile([128, 2 * NTH], F32, tag="sums", bufs=3)
                nc.vector.tensor_add(
                    stq[:, 0:NTH], acc[0][:, t0:t1], acc[1][:, t0:t1]
                )
                for i in range(t0, t1):
                    scr = sb.tile([128, 128], F32, tag="scr")
                    nc.vector.scalar_tensor_tensor(
                        scr[:],
                        y_b[:, 128 * i : 128 * i + 128],
                        1.0,
                        y_b[:, 128 * i : 128 * i + 128],
                        A.mult,
                        A.mult,
                        accum_out=stq[:, NTH + i - t0 : NTH + i - t0 + 1],
                    )
                nc.sync.dma_start(
                    out=stats_in[b, ch].rearrange("c p t -> p c t"),
                    in_=stq[:].rearrange("p (c t) -> p c t", t=NTH),
                )
                if fake_ar:
                    nc.sync.dma_start(out=stats_out[b, ch], in_=stats_in[b, ch])
                else:
                    nc.gpsimd.collective_compute(
                        "AllReduce",
                        A.add,
                        replica_groups=rg,
                        ins=[stats_in[b, ch].opt()],
                        outs=[stats_out[b, ch].opt()],
                    )

            def emit_ln(b, ch, y_b, split=False):
                t0 = ch * NTH
                red = sb.tile([128, 2 * NTH], F32, tag="red", bufs=3)
                nc.sync.dma_start(
                    out=red[:].rearrange("p (c t) -> p c t", t=NTH),
                    in_=stats_out[b, ch].rearrange("c p t -> p c t"),
                )
                mean = sb.tile([128, NTH], F32, tag="mean", bufs=3)
                nc.vector.tensor_scalar(
                    mean[:], red[:, 0:NTH], 1.0 / D, None, A.mult
                )
                msq = sb.tile([128, NTH], F32, tag="msq", bufs=3)
                nc.vector.tensor_mul(msq[:], mean[:], mean[:])
                var = sb.tile([128, NTH], F32, tag="var", bufs=3)
                nc.vector.scalar_tensor_tensor(
                    var[:], red[:, NTH : 2 * NTH], 1.0 / D, msq[:], A.mult,
                    A.subtract,
                )
                lnv = sb.tile([128, NTH], F32, tag="lnv", bufs=3)
                nc.scalar.activation(lnv[:], var[:], AF.Ln, bias=eps_t[:])
                rstd = sb.tile([128, NTH], F32, tag="rstd", bufs=3)
                nc.scalar.activation(rstd[:], lnv[:], AF.Exp, scale=-0.5)
                ostb = sb.tile([128, 128 * NTH], F32, tag="ost", bufs=2)
                eng = nc.gpsimd if ((b * NHALF + ch) % 2 == 0 and b < B_ - 1) else nc.sync
                hn = NTH // 2
                for k in range(NTH):
                    i = t0 + k
                    nc.vector.tensor_scalar(
                        ostb[:, 128 * k : 128 * k + 128],
                        y_b[:, 128 * i : 128 * i + 128],
                        mean[:, k : k + 1],
                        rstd[:, k : k + 1],
                        A.subtract,
                        A.mult,
                    )
                    if apply_affine:
                        nc.vector.tensor_mul(
                            ostb[:, 128 * k : 128 * k + 128],
                            ostb[:, 128 * k : 128 * k + 128],
                            gam_t[:],
                        )
                        nc.vector.tensor_add(
                            ostb[:, 128 * k : 128 * k + 128],
                            ostb[:, 128 * k : 128 * k + 128],
                            bet_t[:],
                        )
                    if split and k == hn - 1:
                        # first-half store overlaps the remaining normalizes
                        eng.dma_start(
                            out=out[b, 128 * t0 : 128 * (t0 + hn), :].rearrange(
                                "(i p) d -> p i d", p=128
                            ),
                            in_=ostb[:, 0 : 128 * hn].rearrange(
                                "p (i d) -> p i d", d=128
                            ),
                        )
                lo_t = t0 + hn if split else t0
                eng.dma_start(
                    out=out[b, 128 * lo_t : 128 * (t0 + NTH), :].rearrange(
                        "(i p) d -> p i d", p=128
                    ),
                    in_=ostb[:, 128 * (lo_t - t0) :].rearrange(
                        "p (i d) -> p i d", d=128
                    ),
                )

            y_tiles = {}
            bstate = {}
            pstate = {}
            pw = min(1024, S_)
            NP = B_ * HPC

            def emit_proj(pair):
                b, hh = divmod(pair, HPC)
                if hh == 0:
                    if b == 0:
                        xth = xth0
                    else:
                        xth = [None, None]
                        for h2 in range(HPC):
                            xth[h2] = sb.tile(
                                [DK + 1, S_], BF, tag="xth", name=f"xth{b}_{h2}", bufs=B_ * HPC
                            )
                            eng = nc.sync if h2 == 0 else nc.gpsimd
                            eng.dma_start(
                                out=xth[h2][:, 0 : S_ // 2], in_=xt[b, h2, :, 0 : S_ // 2]
                            )
                            eng.dma_start(
                                out=xth[h2][:, S_ // 2 : S_], in_=xt[b, h2, :, S_ // 2 : S_]
                            )
                    xs_b = sb.tile([128, S_], F32, tag="xs", name=f"xs{b}")
                    y_b = sb.tile([128, S_], F32, tag=f"y{b}", name=f"y{b}")
                    y_tiles[b] = y_b
                    bstate[b] = (xth, xs_b, y_b, {})
                    need_xs_dma = True
                else:
                    need_xs_dma = False
                xth, xs_b, y_b, accs = bstate[b]
                xh = xth[hh]
                # z = [M @ xh^T + u | beta-row]: scores become xh_aug^T @ z.
                # Emit per s-half (z chunks then V group) so the first scores
                # only wait on the first half of the x^T DMA.
                z = sb.tile([DK + 1, S_], BF, tag="z", name=f"z{pair}", bufs=NP)
                v = sb.tile([128, NT_ * VW], FP8, tag="v", name=f"v{pair}", bufs=NP)
                v3 = v[:].rearrange("p (t w) -> p t w", w=VW)
                nc.vector.memset(v3[:, :, DK : DK + 1], 1.0)
                gv = min(8, NT_)
                for half in range(NT_ // gv):
                    for c in (2 * half, 2 * half + 1):
                        zp = ps.tile([128, 512], F32, tag="op", bufs=2, name=f"zp{c}")
                        nc.tensor.matmul(
                            zp[0 : DK + 1, :],
                            lhsT=zw_t[:, (DK + 1) * hh : (DK + 1) * (hh + 1)],
                            rhs=xh[0:DK, 512 * c : 512 * c + 512],
                            start=True,
                            stop=True,
                        )
                        nc.vector.tensor_scalar(
                            z[:, 512 * c : 512 * c + 512],
                            zp[0 : DK + 1, :],
                            zb_t[:, hh : hh + 1],
                            None,
                            A.add,
                        )
                    g = half
                    vp = ps.tile([128, 512], F32, tag="op", bufs=2, name=f"vp{g}")
                    for u in range(gv):
                        j = gv * g + u
                        nc.tensor.matmul(
                            vp[:, DK * u : DK * u + DK],
                            lhsT=xh[0:DK, 128 * j : 128 * j + 128],
                            rhs=wv_t[:, hh * DK : hh * DK + DK],
                            start=True,
                            stop=True,
                        )
                    nc.vector.tensor_tensor(
                        v3[:, gv * g : gv * g + gv, 0:DK],
                        vp[:, 0 : gv * DK].rearrange("q (t w) -> q t w", w=DK),
                        bv16_t[:].rearrange("q (h t w) -> q (h t) w", h=HPC, w=DK)[
                            :, hh * 16 : hh * 16 + gv, :
                        ],
                        A.add,
                    )
                if need_xs_dma:
                    nc.sync.dma_start(
                        out=xs_b[:].rearrange("p (i d) -> p i d", d=128),
                        in_=xs[b].rearrange("(i p) d -> p i d", p=128),
                    )
                acc_h = sb.tile([128, NT_], F32, tag=f"acc{hh}", name=f"acc{pair}", bufs=B_)
                accs[hh] = acc_h
                pstate[pair] = (xh, z, v3, acc_h)

            def emit_jhalf(pair, hs, mid_hook=None, service_hook=None, final=False):
                """Score/exp/PV loop for one 1024-col s-half; returns the
                deferred transpose/normalize epilogue closure. PV runs as
                fp8 DoubleRow over PAIRS of key-tiles (j, j+1): V planes are
                adjacent slices of v3, P planes are halves of a shared fp8
                tile — one matmul covers two key-tiles at 0.5 cycles/col."""
                b, hh = divmod(pair, HPC)
                xh, z, v3, acc_h = pstate[pair]
                _, xs_b, y_b, accs = bstate[b]
                he = min(S_, hs + 1024)
                w = he - hs
                nj = he // 128
                npair_t = nj // 2
                opA = ps.tile([DK + 1, 512], F32, tag="op", bufs=2)
                opB = ps.tile([DK + 1, 512], F32, tag="op", bufs=2)
                # last tile-pair touching each 512-col bank (for stop flags)
                last_m = [
                    max(
                        mm
                        for mm in range(npair_t)
                        if max(0, 256 * mm - hs) < 512 * (g + 1)
                    )
                    for g in range(w // 512)
                ]
                ot = sb.tile([DK + 1, 1024], F32, tag="ot", bufs=4)

                def _epi_piece(g, stq):
                    """Inline transpose/normalize + stats fill for one 512-col
                    bank of the FINAL half (tail-latency special case)."""
                    t0 = hs // 128 + 4 * g
                    tp = ps.tile([128, 512], F32, tag="op", bufs=2, name=f"ftp{g}")
                    for k in range(4):
                        nc.tensor.transpose(
                            tp[:, 128 * k : 128 * k + DK + 1],
                            ot[:, 512 * g + 128 * k : 512 * g + 128 * k + 128],
                            idn_t[:],
                        )
                    r4 = sb.tile([128, 4], F32, tag="r8", bufs=3, name=f"fr4{g}")
                    nc.vector.reciprocal(
                        r4[:],
                        tp[:].rearrange("q (k c) -> q k c", c=128)[:, 0:4, DK : DK + 1],
                    )
                    for k in range(4):
                        i = t0 + k
                        nc.vector.scalar_tensor_tensor(
                            y_b[:, 128 * i + DK * hh : 128 * i + DK * hh + DK],
                            tp[:, 128 * k : 128 * k + DK],
                            r4[:, k : k + 1],
                            xs_b[:, 128 * i + DK * hh : 128 * i + DK * hh + DK],
                            A.mult,
                            A.add,
                            accum_out=acc_h[:, i : i + 1],
                        )
                    nc.vector.tensor_add(
                        stq[:, 4 * g : 4 * g + 4],
                        accs[0][:, t0 : t0 + 4],
                        acc_h[:, t0 : t0 + 4],
                    )
                    for k in range(4):
                        i = t0 + k
                        scr = sb.tile([128, 128], F32, tag="scr")
                        nc.vector.scalar_tensor_tensor(
                            scr[:],
                            y_b[:, 128 * i : 128 * i + 128],
                            1.0,
                            y_b[:, 128 * i : 128 * i + 128],
                            A.mult,
                            A.mult,
                            accum_out=stq[:, NTH + 4 * g + k : NTH + 4 * g + k + 1],
                        )

                stq_f = None
                if final:
                    stq_f = sb.tile([128, 2 * NTH], F32, tag="sums", bufs=3, name="stqf")

                prev_pv = None
                for m in range(npair_t):
                    p2m = sb.tile(
                        [128, 2048], FP8, tag="p", bufs=PBUFS, name=f"p2_{m}"
                    )
                    p3 = p2m[:].rearrange("q (i c) -> q i c", c=1024)
                    los = [0, 0]
                    for par in range(2):
                        j = 2 * m + par
                        s0 = 128 * j
                        rel = s0 - hs
                        sp = ps.tile([128, 1024], F32, tag="sp", bufs=SPBUFS)
                        if rel < 0:
                            ss = 0
                            while ss < w:
                                sl = min(512, w - ss)
                                nc.tensor.matmul(
                                    sp[:, ss : ss + sl],
                                    lhsT=xh[:, s0 : s0 + 128],
                                    rhs=z[:, hs + ss : hs + ss + sl],
                                    start=True,
                                    stop=True,
                                )
                                ss += sl
                            lo = 0
                        else:
                            lo = rel
                            nc.tensor.matmul(
                                sp[:, rel : rel + 128],
                                lhsT=idn128_t,
                                rhs=maskt_t,
                                start=True,
                                stop=False,
                                perf_mode=mybir.MatmulPerfMode.DoubleRow,
                                skip_group_check=True,
                            )
                            nc.tensor.matmul(
                                sp[:, rel : rel + 128],
                                lhsT=xh[:, s0 : s0 + 128],
                                rhs=z[:, s0 : s0 + 128],
                                start=False,
                                stop=True,
                                skip_group_check=True,
                            )
                            ss = rel + 128
                            while ss < w:
                                sl = min(512 - (ss % 512), w - ss)
                                nc.tensor.matmul(
                                    sp[:, ss : ss + sl],
                                    lhsT=xh[:, s0 : s0 + 128],
                                    rhs=z[:, hs + ss : hs + ss + sl],
                                    start=True,
                                    stop=True,
                                )
                                ss += sl
                        los[par] = lo
                        nc.scalar.activation(p3[:, par, lo:w], sp[:, lo:w], AF.Exp)
                        if j == min(HOOKJ, nj - 1) and mid_hook is not None:
                            mid_hook()
                    if m in (1, 4) and service_hook is not None:
                        service_hook()

                    # PV for the pair, deferred by one pair so the PE computes
                    # the next scores while ACT exps this pair
                    def _pv(m=m, p3=p3, lo0=los[0], lo1=los[1]):
                        j0 = 2 * m
                        # ragged diagonal: [lo0, lo1) has only plane 0 valid
                        cs = lo0
                        while cs < lo1:
                            ce = min(512 * (cs // 512) + 512, lo1)
                            opt = opA if cs < 512 else opB
                            nc.tensor.matmul(
                                opt[:, cs % 512 : cs % 512 + (ce - cs)],
                                lhsT=v3[:, j0, 0 : DK + 1],
                                rhs=p3[:, 0, cs:ce],
                                start=(m == 0),
                                stop=False,
                                skip_group_check=True,
                            )
                            cs = ce
                        # both planes valid: DoubleRow over (j0, j0+1)
                        cs = lo1
                        while cs < w:
                            ce = min(512 * (cs // 512) + 512, w)
                            g = cs // 512
                            opt = opA if cs < 512 else opB
                            first = (m == 0) and (cs == lo1 == lo0 or cs >= 512)
                            nc.tensor.matmul(
                                opt[:, cs % 512 : cs % 512 + (ce - cs)],
                                lhsT=v3[:, j0 : j0 + 2, 0 : DK + 1],
                                rhs=p3[:, :, cs:ce],
                                start=first,
                                stop=(m == last_m[g] and ce == min(w, 512 * (g + 1))),
                                perf_mode=mybir.MatmulPerfMode.DoubleRow,
                                skip_group_check=True,
                            )
                            cs = ce

                    if prev_pv is not None:
                        prev_pv()
                        if final and m - 1 == last_m[0]:
                            # bank 0 fully accumulated: drain + normalize it
                            # while the tail PVs of bank 1 still run
                            nc.vector.tensor_copy(ot[:, 0:512], opA[:, 0:512])
                            _epi_piece(0, stq_f)
                    prev_pv = _pv
                if prev_pv is not None:
                    prev_pv()
                if not final:
                    nc.vector.tensor_copy(ot[:, 0:512], opA[:, 0:512])
                nc.vector.tensor_copy(ot[:, 512:w], opB[:, 0 : w - 512])
                if final:
                    _epi_piece(1, stq_f)
                    nc.sync.dma_start(
                        out=stats_in[b, 1].rearrange("c p t -> p c t"),
                        in_=stq_f[:].rearrange("p (c t) -> p c t", t=NTH),
                    )
                    if fake_ar:
                        nc.sync.dma_start(out=stats_out[b, 1], in_=stats_in[b, 1])
                    else:
                        nc.gpsimd.collective_compute(
                            "AllReduce",
                            A.add,
                            replica_groups=rg,
                            ins=[stats_in[b, 1].opt()],
                            outs=[stats_out[b, 1].opt()],
                        )
                    emit_ln(b, 1, y_b)
                    return None

                def _epilogue():
                    nk = he // 128 - hs // 128
                    tps = [
                        ps.tile([128, 512], F32, tag="op", bufs=2, name=f"tp{g}")
                        for g in range((nk + 3) // 4)
                    ]
                    for i in range(hs // 128, he // 128):
                        k = i - hs // 128
                        tp = tps[k // 4]
                        nc.tensor.transpose(
                            tp[:, 128 * (k % 4) : 128 * (k % 4) + DK + 1],
                            ot[:, 128 * i - hs : 128 * i - hs + 128],
                            idn_t[:],
                        )
                    r8 = sb.tile([128, 8], F32, tag="r8", bufs=3)
                    for g, tp in enumerate(tps):
                        gn = min(4, nk - 4 * g)
                        nc.vector.reciprocal(
                            r8[:, 4 * g : 4 * g + gn],
                            tp[:].rearrange("q (k c) -> q k c", c=128)[
                                :, 0:gn, DK : DK + 1
                            ],
                        )
                    for i in range(hs // 128, he // 128):
                        k = i - hs // 128
                        tp = tps[k // 4]
                        nc.vector.scalar_tensor_tensor(
                            y_b[:, 128 * i + DK * hh : 128 * i + DK * hh + DK],
                            tp[:, 128 * (k % 4) : 128 * (k % 4) + DK],
                            r8[:, k : k + 1],
                            xs_b[:, 128 * i + DK * hh : 128 * i + DK * hh + DK],
                            A.mult,
                            A.add,
                            accum_out=acc_h[:, i : i + 1],
                        )
                    if hh == HPC - 1:
                        _emit_stats(b, y_b, accs, hs // 1024)
                        ln_ready.append((b, hs // 1024))

                return _epilogue

            emit_proj(0)
            pending = []
            ln_ready = []
            for pair in range(NP):
                for k, hs in enumerate(range(0, S_, 1024)):
                    hook = None
                    if k == 0 and pair + 1 < NP:
                        hook = (lambda pr=pair: emit_proj(pr + 1))
                    svc = None
                    if pair == NP - 1 and k == 1:
                        # pop the last pair's h0 epilogue mid-h1 so its LN
                        # stats chain overlaps the remaining exp tiles
                        svc = lambda: pending.pop(0)() if pending else None
                    epi = emit_jhalf(pair, hs, mid_hook=hook, service_hook=svc)
                    if epi is not None:
                        pending.append(epi)
                    if len(pending) > EPI_LAG:
                        pending.pop(0)()
                    if pair >= NP - 2:
                        # drain earlier batches' LN during the last two pairs
                        # so only the final batch's LN remains in the tail
                        for _ in range(2):
                            if ln_ready:
                                bb, cc = ln_ready.pop(0)
                                emit_ln(bb, cc, y_tiles[bb])
            for e in pending:
                e()
            while ln_ready:
                bb, cc = ln_ready.pop(0)
                emit_ln(bb, cc, y_tiles[bb], split=not ln_ready)


    # Restrict Exp/Ln to the shared natural_log_exp_and_others table set so
    # the whole kernel uses one ACT table load (indices preserved).
    import concourse.bacc as _bacc_mod

    _orig_tables = _bacc_mod.get_activation_tables

    def _filtered_tables(arch):
        out = {}
        for name, fns in _orig_tables(arch).items():
            if name != "natural_log_exp_and_others":
                fns = set(fns) - {AF.Exp, AF.Ln}
            out[name] = fns
        return out

    _bacc_mod.get_activation_tables = _filtered_tables
    try:
        nc.compile()
    finally:
        _bacc_mod.get_activation_tables = _orig_tables
    return nc


_GRAPH_CACHE = {}


def _get_graph(apply_affine: bool) -> bass.Bass:
    if apply_affine not in _GRAPH_CACHE:
        _GRAPH_CACHE[apply_affine] = _build_graph(apply_affine)
    return _GRAPH_CACHE[apply_affine]


def _prep_in_maps(x, Wq, bq, Wk, bk, Wv, bv, gamma, beta, apply_affine):
    scale = 1.0 / np.sqrt(np.float32(DK))
    in_maps = []
    for i in range(NCORES):
        dsl = slice(DC * i, DC * (i + 1))
        hsl = slice(HPC * i, HPC * (i + 1))
        x_sl = x[:, :, dsl]
        xt_full = x_sl.transpose(0, 2, 1).reshape(x.shape[0], HPC, DK, x.shape[1])
        xt_aug = np.concatenate(
            [xt_full, np.ones((x.shape[0], HPC, 1, x.shape[1]), np.float32)], axis=2
        )
        Wq_s = (Wq[hsl] * scale).astype(np.float64)
        bq_s = (bq[hsl] * scale).astype(np.float64)
        Wk_h = Wk[hsl].astype(np.float64)
        bk_h = bk[hsl].astype(np.float64)
        M = np.einsum("hde,hfe->hdf", Wk_h, Wq_s)      # [h, dK, dQ]
        u = np.einsum("hde,he->hd", Wk_h, bq_s)        # alpha coeffs (per t)
        wvec = np.einsum("hde,he->hd", Wq_s, bk_h)     # beta coeffs (per s)
        cconst = np.einsum("he,he->h", bk_h, bq_s)
        # lhsT for z: [d', dK | wvec]; z rows 0..63 = M@xh^T + u, row 64 = xh.w + c
        zw_np = np.concatenate(
            [M.transpose(0, 2, 1), wvec[:, :, None]], axis=2
        )  # [h, d'(=dQ... contraction dim), dK+1]
        zb_np = np.concatenate([u, cconst[:, None]], axis=1)[:, :, None]
        m = {
            "xt": np.ascontiguousarray(xt_aug).astype(BF_NP),
            "xs": np.ascontiguousarray(x_sl),
            "wpack": np.ascontiguousarray(
                np.concatenate(
                    [zw_np[0], zw_np[1], Wv[hsl][0], Wv[hsl][1]], axis=1
                )
            ).astype(BF_NP),
            "zb": np.ascontiguousarray(zb_np[:, :, 0].T).astype(np.float32),
            "bv16": np.ascontiguousarray(
                np.tile(bv[hsl][:, None, :], (1, 128, 16))
            ).astype(np.float32),
        }
        if apply_affine:
            m["gam"] = np.ascontiguousarray(
                np.tile(gamma[dsl][None, :], (128, 1))
            ).astype(np.float32)
            m["bet"] = np.ascontiguousarray(
                np.tile(beta[dsl][None, :], (128, 1))
            ).astype(np.float32)
        in_maps.append(m)
    return in_maps


def kernel(x, Wq, bq, Wk, bk, Wv, bv, gamma, beta):
    global LAST_RESULTS
    x = np.asarray(x, np.float32)
    Wq = np.asarray(Wq, np.float32)
    bq = np.asarray(bq, np.float32)
    Wk = np.asarray(Wk, np.float32)
    bk = np.asarray(bk, np.float32)
    Wv = np.asarray(Wv, np.float32)
    bv = np.asarray(bv, np.float32)
    gamma = np.asarray(gamma, np.float32)
    beta = np.asarray(beta, np.float32)

    apply_affine = not (
        np.allclose(gamma, 1.0, atol=0.0, rtol=0.0)
        and np.allclose(beta, 0.0, atol=0.0, rtol=0.0)
    )
    fake_ar = bool(int(os.environ.get("KERNEL_FAKE_AR", "0")))
    nc = _get_graph(apply_affine) if not fake_ar else _build_graph(apply_affine, fake_ar=True)

    in_maps = _prep_in_maps(x, Wq, bq, Wk, bk, Wv, bv, gamma, beta, apply_affine)

    res = run_bass_kernel_spmd(
        nc,
        in_maps,
        core_ids=list(range(NCORES)),
        trace=bool(int(os.environ.get("KERNEL_TRACE", "0"))),
    )
    LAST_RESULTS = res
    outs = [np.asarray(r["out"], np.float32) for r in res.results]
    return np.concatenate(outs, axis=2)


if __name__ == "__main__":
    nc = _build_graph(False)
    print("graph built ok:", len(nc.inst_map), "instructions")



# revision 80
# speedup vs baseline: 1.1025x; 1.0128x over previous
"""Trainium2 Bass kernel: per-head attention + residual + LayerNorm.

Problem shape: x [4, 2048, 1024], 16 heads of dk=64, causal softmax attention
with per-head Q/K/V linear projections, residual add, LayerNorm(D).

Sharding (8 cores): head-parallel. Core i owns heads (2i, 2i+1), i.e. feature
columns 128*i : 128*(i+1). Each core computes its feature slice of the output;
the only cross-core communication is a tiny per-batch AllReduce of LayerNorm
partial sums (sum and sum-of-squares over each core's 128 features). The host
shards/gathers and pre-transposes x (the [dk, S] layout each head needs).

Per-core kernel design (bf16 score matmuls, fp8 DoubleRow PV, fp32 PSUM):
- Scores via a host-precomputed bilinear form: scores^T = xh_aug^T @ z with
  z = [M @ xh^T + u | beta-row], M = Wk (Wq/sqrt(dk))^T. One projected tensor
  (z) instead of Q and K halves the PSUM->SBUF copies; x^T itself (with a
  built-in ones row for the bias terms) is the stationary matmul operand.
- Flash-style t-outer loop over 1024-col query halves; scores accumulate in
  PSUM [128,1024] chunks on a dedicated 3-deep ring (6 banks) that serves
  ONLY the score->exp pipeline; exp runs on ScalarE straight from PSUM,
  writing P as fp8e4m3 into 6-deep [128, 2, 1024] pair tiles.
- PV runs as fp8 DoubleRow over PAIRS of key-tiles: V (fp8, ones-augmented,
  VW-padded) exposes planes as adjacent v3 slices, P planes are the halves
  of the shared pair tile, so one matmul covers two key-tiles at 0.5
  cycles/col (3.4x fewer PE cycles than per-tile bf16 PV). Ragged diagonal
  pairs emit a single-plane prologue for the 128 columns only the even tile
  covers; stop flags close each 512-col PSUM bank at its last touching pair.
- Causal mask: an fp8e5 DoubleRow identity-matmul accumulates -40 onto the
  upper triangle of the diagonal 128-block before exp (64 PE cycles);
  sub-diagonal dead zones are simply never read by PV.
- PV accumulates O^T [65,1024] in PSUM with the ones-augmented V, so softmax
  denominators ride along as row 64; per 128-tile PE-transposes then let a
  single fused DVE op do (O*1/l + x) with the row-sum accumulated for free.
- rstd = exp(-0.5*ln(var+eps)) keeps every activation in one ACT table set
  (natural_log_exp_and_others; enforced by filtering the set map at compile).
- Emission is software-pipelined: the next pair's z/V projection is emitted
  mid-way through the current score loop, the transpose/normalize epilogue is
  deferred one unit, head-1 x^T loads ride the gpsimd DMA queue, LayerNorm
  stats AllReduce per s-half in one fused [sum|sumsq] DMA, earlier batches'
  LN drains during the last two pairs, the last pair's h0 epilogue pops
  mid-h1, and the final output store is split so only the last batch's
  LN chain remains exposed in the tail.

Self-contained: hardcodes all shapes; no sibling imports.
"""

import os
import numpy as np
import ml_dtypes

import concourse.bass as bass
import concourse.bacc as bacc
import concourse.mybir as mybir
from concourse.tile import TileContext
from concourse.bass_utils import run_bass_kernel_spmd

B, S, D, H = 4, 2048, 1024, 16
NCORES = 8
HPC = H // NCORES          # heads per core = 2
DK = D // H                # 64
DC = HPC * DK              # 128 feature cols per core
NT = S // 128              # 16 row tiles of 128
EPS = 1e-5
MASKNEG = -40.0
SPBUFS = 3
OPBUFS = 1
EPI_LAG = 1
HOOKJ = 6
QKBUFS = 3
PBUFS = 6
BF = mybir.dt.bfloat16
F32 = mybir.dt.float32
FP8 = mybir.dt.float8e4
FP8E5 = mybir.dt.float8e5
BF_NP = ml_dtypes.bfloat16
FP8_NP = ml_dtypes.float8_e4m3
FP8E5_NP = ml_dtypes.float8_e5m2
VW = 80  # padded V-tile row stride (65 used) so DoubleRow plane step is 16B-aligned
RG = [list(range(NCORES))]
A = mybir.AluOpType
AF = mybir.ActivationFunctionType

LAST_RESULTS = None  # BassKernelResults of the last run (for test harness)


def _build_graph(apply_affine: bool, B_: int = B, S_: int = S, rg=None, fake_ar: bool = False) -> bass.Bass:
    nc = bacc.Bacc()
    NT_ = S_ // 128
    if rg is None:
        rg = RG

    xt = nc.declare_dram_parameter("xt", [B_, HPC, DK + 1, S_], BF, isOutput=False)
    xs = nc.declare_dram_parameter("xs", [B_, S_, DC], F32, isOutput=False)
    wpack = nc.declare_dram_parameter(
        "wpack", [DK, HPC * (DK + 1) + HPC * DK], BF, isOutput=False
    )
    zb = nc.declare_dram_parameter("zb", [DK + 1, HPC], F32, isOutput=False)
    bv16 = nc.declare_dram_parameter("bv16", [HPC, 128, 16 * DK], F32, isOutput=False)
    if apply_affine:
        gam = nc.declare_dram_parameter("gam", [128, DC], F32, isOutput=False)
        bet = nc.declare_dram_parameter("bet", [128, DC], F32, isOutput=False)
    out = nc.declare_dram_parameter("out", [B_, S_, DC], F32, isOutput=True)

    # constants baked into the NEFF
    idn_h = nc.inline_tensor(np.eye(DK + 1, dtype=np.float32), name="idn")
    trineg_np = np.where(
        np.arange(128)[:, None] > np.arange(128)[None, :], MASKNEG, 0.0
    ).astype(np.float32)
    # DoubleRow-plane layout: [64, 2, 128] -> [64, 256], plane i = rows 64i..64i+63
    idn2_np = np.eye(128, dtype=np.float32).reshape(2, 64, 128).transpose(1, 0, 2)
    trineg2_np = trineg_np.reshape(2, 64, 128).transpose(1, 0, 2)
    imask_h = nc.inline_tensor(
        np.concatenate([idn2_np, trineg2_np], axis=2).reshape(64, 512).astype(
            FP8E5_NP
        ),
        name="imask",
    )

    # collective bounce buffers: LayerNorm stats per (batch, s-half):
    # [2(sum,sumsq), 128 rows, tiles-in-half]
    NHALF = (S_ + 1023) // 1024
    NTH = NT_ // NHALF
    stats_in = nc.dram_tensor("stats_in", [B_, NHALF, 2, 128, NTH], F32)
    stats_out = nc.dram_tensor(
        "stats_out", [B_, NHALF, 2, 128, NTH], F32, addr_space="Shared"
    )

    with TileContext(nc) as tc:
        with (
            tc.tile_pool(name="consts", bufs=1) as cpool,
            tc.tile_pool(name="sb", bufs=2) as sb,
            tc.tile_pool(name="ps", bufs=1, space="PSUM") as ps,
        ):
            # ---- load constants (single coalesced DMAs, first-needed first) ----
            wp_t = cpool.tile([DK, HPC * (DK + 1) + HPC * DK], BF, tag="wp")
            nc.sync.dma_start(out=wp_t[:], in_=wpack[:, :])
            zw_t = wp_t[:][:, 0 : HPC * (DK + 1)]
            wv_t = wp_t[:][:, HPC * (DK + 1) : HPC * (DK + 1) + HPC * DK]
            # queue order tuned for the first exp: wpack, then the first
            # half of head-0 x^T (all the first projection needs), then the
            # small constants the first z-copy and diagonal mask need, then
            # the rest of batch-0 x^T
            xth0 = [
                sb.tile([DK + 1, S_], BF, tag="xth", name=f"xth0_{h2}", bufs=B_ * HPC)
                for h2 in range(HPC)
            ]
            nc.sync.dma_start(out=xth0[0][:, 0 : S_ // 2], in_=xt[0, 0, :, 0 : S_ // 2])
            zbq_t = cpool.tile([DK + 1, HPC], F32, tag="zb")
            nc.sync.dma_start(out=zbq_t[:], in_=zb[:, :])
            imaskq_t = cpool.tile([64, 512], FP8E5, tag="imask")
            nc.sync.dma_start(out=imaskq_t[:], in_=imask_h[:, :])
            nc.sync.dma_start(out=xth0[0][:, S_ // 2 : S_], in_=xt[0, 0, :, S_ // 2 : S_])
            # head-1 x^T on the gpsimd queue: overlaps head-0's sync-queue
            # transfers, and nothing before the h0 scores needs it
            nc.gpsimd.dma_start(out=xth0[1][:, 0 : S_ // 2], in_=xt[0, 1, :, 0 : S_ // 2])
            nc.gpsimd.dma_start(out=xth0[1][:, S_ // 2 : S_], in_=xt[0, 1, :, S_ // 2 : S_])
            idn_t = cpool.tile([DK + 1, DK + 1], F32, tag="idn")
            nc.gpsimd.dma_start(out=idn_t[:], in_=idn_h[:, :])
            bv16_t = cpool.tile([128, HPC * 16 * DK], F32, tag="bv16")
            for h in range(HPC):
                nc.gpsimd.dma_start(
                    out=bv16_t[:, 16 * DK * h : 16 * DK * (h + 1)], in_=bv16[h]
                )
            if apply_affine:
                gam_t = cpool.tile([128, DC], F32, tag="gam")
                nc.sync.dma_start(out=gam_t[:], in_=gam[:, :])
                bet_t = cpool.tile([128, DC], F32, tag="bet")
                nc.sync.dma_start(out=bet_t[:], in_=bet[:, :])

            zb_t = zbq_t
            imask3 = imaskq_t[:].rearrange("p (i c) -> p i c", c=256)
            idn128_t = imask3[:, :, 0:128]
            maskt_t = imask3[:, :, 128:256]
            eps_t = cpool.tile([128, 1], F32, tag="eps")
            nc.vector.memset(eps_t[:], EPS)

            def _emit_stats(b, y_b, acc, ch):
                # LayerNorm partial stats + AllReduce for one s-half.
                # One fused [sum|sumsq] tile -> one DMA.
                t0, t1 = ch * NTH, ch * NTH + NTH
                stq = sb.tile([128, 2 * NTH], F32, tag="sums", bufs=3)
                nc.vector.tensor_add(
                    stq[:, 0:NTH], acc[0][:, t0:t1], acc[1][:, t0:t1]
                )
                for i in range(t0, t1):
                    scr = sb.tile([128, 128], F32, tag="scr")
                    nc.vector.scalar_tensor_tensor(
                        scr[:],
                        y_b[:, 128 * i : 128 * i + 128],
                        1.0,
                        y_b[:, 128 * i : 128 * i + 128],
                        A.mult,
                        A.mult,
                        accum_out=stq[:, NTH + i - t0 : NTH + i - t0 + 1],
                    )
                nc.sync.dma_start(
                    out=stats_in[b, ch].rearrange("c p t -> p c t"),
                    in_=stq[:].rearrange("p (c t) -> p c t", t=NTH),
                )
                if fake_ar:
                    nc.sync.dma_start(out=stats_out[b, ch], in_=stats_in[b, ch])
                else:
                    nc.gpsimd.collective_compute(
                        "AllReduce",
                        A.add,
                        replica_groups=rg,
                        ins=[stats_in[b, ch].opt()],
                        outs=[stats_out[b, ch].opt()],
                    )

            def emit_ln(b, ch, y_b, split=False):
                t0 = ch * NTH
                red = sb.tile([128, 2 * NTH], F32, tag="red", bufs=3)
                nc.sync.dma_start(
                    out=red[:].rearrange("p (c t) -> p c t", t=NTH),
                    in_=stats_out[b, ch].rearrange("c p t -> p c t"),
                )
                mean = sb.tile([128, NTH], F32, tag="mean", bufs=3)
                nc.vector.tensor_scalar(
                    mean[:], red[:, 0:NTH], 1.0 / D, None, A.mult
                )
                msq = sb.tile([128, NTH], F32, tag="msq", bufs=3)
                nc.vector.tensor_mul(msq[:], mean[:], mean[:])
                var = sb.tile([128, NTH], F32, tag="var", bufs=3)
                nc.vector.scalar_tensor_tensor(
                    var[:], red[:, NTH : 2 * NTH], 1.0 / D, msq[:], A.mult,
                    A.subtract,
                )
                lnv = sb.tile([128, NTH], F32, tag="lnv", bufs=3)
                nc.scalar.activation(lnv[:], var[:], AF.Ln, bias=eps_t[:])
                rstd = sb.tile([128, NTH], F32, tag="rstd", bufs=3)
                nc.scalar.activation(rstd[:], lnv[:], AF.Exp, scale=-0.5)
                ostb = sb.tile([128, 128 * NTH], F32, tag="ost", bufs=2)
                eng = nc.gpsimd if ((b * NHALF + ch) % 2 == 0 and b < B_ - 1) else nc.sync
                hn = NTH // 2
                for k in range(NTH):
                    i = t0 + k
                    nc.vector.tensor_scalar(
                        ostb[:, 128 * k : 128 * k + 128],
                        y_b[:, 128 * i : 128 * i + 128],
                        mean[:, k : k + 1],
                        rstd[:, k : k + 1],
                        A.subtract,
                        A.mult,
                    )
                    if apply_affine:
                        nc.vector.tensor_mul(
                            ostb[:, 128 * k : 128 * k + 128],
                            ostb[:, 128 * k : 128 * k + 128],
                            gam_t[:],
                        )
                        nc.vector.tensor_add(
                            ostb[:, 128 * k : 128 * k + 128],
                            ostb[:, 128 * k : 128 * k + 128],
                            bet_t[:],
                        )
                    if split and k == hn - 1:
                        # first-half store overlaps the remaining normalizes
                        eng.dma_start(
                            out=out[b, 128 * t0 : 128 * (t0 + hn), :].rearrange(
                                "(i p) d -> p i d", p=128
                            ),
                            in_=ostb[:, 0 : 128 * hn].rearrange(
                                "p (i d) -> p i d", d=128
                            ),
                        )
                lo_t = t0 + hn if split else t0
                eng.dma_start(
                    out=out[b, 128 * lo_t : 128 * (t0 + NTH), :].rearrange(
                        "(i p) d -> p i d", p=128
                    ),
                    in_=ostb[:, 128 * (lo_t - t0) :].rearrange(
                        "p (i d) -> p i d", d=128
                    ),
                )

            y_tiles = {}
            bstate = {}
            pstate = {}
            pw = min(1024, S_)
            NP = B_ * HPC

            def emit_proj(pair):
                b, hh = divmod(pair, HPC)
                if hh == 0:
                    if b == 0:
                        xth = xth0
                    else:
                        xth = [None, None]
                        for h2 in range(HPC):
                            xth[h2] = sb.tile(
                                [DK + 1, S_], BF, tag="xth", name=f"xth{b}_{h2}", bufs=B_ * HPC
                            )
                            eng = nc.sync if h2 == 0 else nc.gpsimd
                            eng.dma_start(
                                out=xth[h2][:, 0 : S_ // 2], in_=xt[b, h2, :, 0 : S_ // 2]
                            )
                            eng.dma_start(
                                out=xth[h2][:, S_ // 2 : S_], in_=xt[b, h2, :, S_ // 2 : S_]
                            )
                    xs_b = sb.tile([128, S_], F32, tag="xs", name=f"xs{b}")
                    y_b = sb.tile([128, S_], F32, tag=f"y{b}", name=f"y{b}")
                    y_tiles[b] = y_b
                    bstate[b] = (xth, xs_b, y_b, {})
                    need_xs_dma = True
                else:
                    need_xs_dma = False
                xth, xs_b, y_b, accs = bstate[b]
                xh = xth[hh]
                # z = [M @ xh^T + u | beta-row]: scores become xh_aug^T @ z.
                # Emit per s-half (z chunks then V group) so the first scores
                # only wait on the first half of the x^T DMA.
                z = sb.tile([DK + 1, S_], BF, tag="z", name=f"z{pair}", bufs=NP)
                v = sb.tile([128, NT_ * VW], FP8, tag="v", name=f"v{pair}", bufs=NP)
                v3 = v[:].rearrange("p (t w) -> p t w", w=VW)
                nc.vector.memset(v3[:, :, DK : DK + 1], 1.0)
                gv = min(8, NT_)
                for half in range(NT_ // gv):
                    for c in (2 * half, 2 * half + 1):
                        zp = ps.tile([128, 512], F32, tag="op", bufs=2, name=f"zp{c}")
                        nc.tensor.matmul(
                            zp[0 : DK + 1, :],
                            lhsT=zw_t[:, (DK + 1) * hh : (DK + 1) * (hh + 1)],
                            rhs=xh[0:DK, 512 * c : 512 * c + 512],
                            start=True,
                            stop=True,
                        )
                        nc.vector.tensor_scalar(
                            z[:, 512 * c : 512 * c + 512],
                            zp[0 : DK + 1, :],
                            zb_t[:, hh : hh + 1],
                            None,
                            A.add,
                        )
                    g = half
                    vp = ps.tile([128, 512], F32, tag="op", bufs=2, name=f"vp{g}")
                    for u in range(gv):
                        j = gv * g + u
                        nc.tensor.matmul(
                            vp[:, DK * u : DK * u + DK],
                            lhsT=xh[0:DK, 128 * j : 128 * j + 128],
                            rhs=wv_t[:, hh * DK : hh * DK + DK],
                            start=True,
                            stop=True,
                        )
                    nc.vector.tensor_tensor(
                        v3[:, gv * g : gv * g + gv, 0:DK],
                        vp[:, 0 : gv * DK].rearrange("q (t w) -> q t w", w=DK),
                        bv16_t[:].rearrange("q (h t w) -> q (h t) w", h=HPC, w=DK)[
                            :, hh * 16 : hh * 16 + gv, :
                        ],
                        A.add,
                    )
                if need_xs_dma:
                    nc.sync.dma_start(
                        out=xs_b[:].rearrange("p (i d) -> p i d", d=128),
                        in_=xs[b].rearrange("(i p) d -> p i d", p=128),
                    )
                acc_h = sb.tile([128, NT_], F32, tag=f"acc{hh}", name=f"acc{pair}", bufs=B_)
                accs[hh] = acc_h
                pstate[pair] = (xh, z, v3, acc_h)

            def emit_jhalf(pair, hs, mid_hook=None, service_hook=None, final=False):
                """Score/exp/PV loop for one 1024-col s-half; returns the
                deferred transpose/normalize epilogue closure. PV runs as
                fp8 DoubleRow over PAIRS of key-tiles (j, j+1): V planes are
                adjacent slices of v3, P planes are halves of a shared fp8
                tile — one matmul covers two key-tiles at 0.5 cycles/col."""
                b, hh = divmod(pair, HPC)
                xh, z, v3, acc_h = pstate[pair]
                _, xs_b, y_b, accs = bstate[b]
                he = min(S_, hs + 1024)
                w = he - hs
                nj = he // 128
                npair_t = nj // 2
                opA = ps.tile([DK + 1, 512], F32, tag="op", bufs=2)
                opB = ps.tile([DK + 1, 512], F32, tag="op", bufs=2)
                # last tile-pair touching each 512-col bank (for stop flags)
                last_m = [
                    max(
                        mm
                        for mm in range(npair_t)
                        if max(0, 256 * mm - hs) < 512 * (g + 1)
                    )
                    for g in range(w // 512)
                ]
                ot = sb.tile([DK + 1, 1024], F32, tag="ot", bufs=4)

                def _epi_piece(g, stq):
                    """Inline transpose/normalize + stats fill for one 512-col
                    bank of the FINAL half (tail-latency special case)."""
                    t0 = hs // 128 + 4 * g
                    tp = ps.tile([128, 512], F32, tag="op", bufs=2, name=f"ftp{g}")
                    for k in range(4):
                        nc.tensor.transpose(
                            tp[:, 128 * k : 128 * k + DK + 1],
                            ot[:, 512 * g + 128 * k : 512 * g + 128 * k + 128],
                            idn_t[:],
                        )
                    r4 = sb.tile([128, 4], F32, tag="r8", bufs=3, name=f"fr4{g}")
                    nc.vector.reciprocal(
                        r4[:],
                        tp[:].rearrange("q (k c) -> q k c", c=128)[:, 0:4, DK : DK + 1],
                    )
                    for k in range(4):
                        i = t0 + k
                        nc.vector.scalar_tensor_tensor(
                            y_b[:, 128 * i + DK * hh : 128 * i + DK * hh + DK],
                            tp[:, 128 * k : 128 * k + DK],
                            r4[:, k : k + 1],
                            xs_b[:, 128 * i + DK * hh : 128 * i + DK * hh + DK],
                            A.mult,
                            A.add,
                            accum_out=acc_h[:, i : i + 1],
                        )
                    nc.vector.tensor_add(
                        stq[:, 4 * g : 4 * g + 4],
                        accs[0][:, t0 : t0 + 4],
                        acc_h[:, t0 : t0 + 4],
                    )
                    for k in range(4):
                        i = t0 + k
                        scr = sb.tile([128, 128], F32, tag="scr")
                        nc.vector.scalar_tensor_tensor(
                            scr[:],
                            y_b[:, 128 * i : 128 * i + 128],
                            1.0,
                            y_b[:, 128 * i : 128 * i + 128],
                            A.mult,
                            A.mult,
                            accum_out=stq[:, NTH + 4 * g + k : NTH + 4 * g + k + 1],
                        )

                stq_f = None
                if final:
                    stq_f = sb.tile([128, 2 * NTH], F32, tag="sums", bufs=3, name="stqf")

                prev_pv = None
                for m in range(npair_t):
                    p2m = sb.tile(
                        [128, 2048], FP8, tag="p", bufs=PBUFS, name=f"p2_{m}"
                    )
                    p3 = p2m[:].rearrange("q (i c) -> q i c", c=1024)
                    los = [0, 0]
                    for par in range(2):
                        j = 2 * m + par
                        s0 = 128 * j
                        rel = s0 - hs
                        sp = ps.tile([128, 1024], F32, tag="sp", bufs=SPBUFS)
                        if rel < 0:
                            ss = 0
                            while ss < w:
                                sl = min(512, w - ss)
                                nc.tensor.matmul(
                                    sp[:, ss : ss + sl],
                                    lhsT=xh[:, s0 : s0 + 128],
                                    rhs=z[:, hs + ss : hs + ss + sl],
                                    start=True,
                                    stop=True,
                                )
                                ss += sl
                            lo = 0
                        else:
                            lo = rel
                            nc.tensor.matmul(
                                sp[:, rel : rel + 128],
                                lhsT=idn128_t,
                                rhs=maskt_t,
                                start=True,
                                stop=False,
                                perf_mode=mybir.MatmulPerfMode.DoubleRow,
                                skip_group_check=True,
                            )
                            nc.tensor.matmul(
                                sp[:, rel : rel + 128],
                                lhsT=xh[:, s0 : s0 + 128],
                                rhs=z[:, s0 : s0 + 128],
                                start=False,
                                stop=True,
                                skip_group_check=True,
                            )
                            ss = rel + 128
                            while ss < w:
                                sl = min(512 - (ss % 512), w - ss)
                                nc.tensor.matmul(
                                    sp[:, ss : ss + sl],
                                    lhsT=xh[:, s0 : s0 + 128],
                                    rhs=z[:, hs + ss : hs + ss + sl],
                                    start=True,
                                    stop=True,
                                )
                                ss += sl
                        los[par] = lo
                        nc.scalar.activation(p3[:, par, lo:w], sp[:, lo:w], AF.Exp)
                        if j == min(HOOKJ, nj - 1) and mid_hook is not None:
                            mid_hook()
                    if m in (1, 4) and service_hook is not None:
                        service_hook()

                    # PV for the pair, deferred by one pair so the PE computes
                    # the next scores while ACT exps this pair
                    def _pv(m=m, p3=p3, lo0=los[0], lo1=los[1]):
                        j0 = 2 * m
                        # ragged diagonal: [lo0, lo1) has only plane 0 valid
                        cs = lo0
                        while cs < lo1:
                            ce = min(512 * (cs // 512) + 512, lo1)
                            opt = opA if cs < 512 else opB
                            nc.tensor.matmul(
                                opt[:, cs % 512 : cs % 512 + (ce - cs)],
                                lhsT=v3[:, j0, 0 : DK + 1],
                                rhs=p3[:, 0, cs:ce],
                                start=(m == 0),
                                stop=False,
                                skip_group_check=True,
                            )
                            cs = ce
                        # both planes valid: DoubleRow over (j0, j0+1)
                        cs = lo1
                        while cs < w:
                            ce = min(512 * (cs // 512) + 512, w)
                            g = cs // 512
                            opt = opA if cs < 512 else opB
                            first = (m == 0) and (cs == lo1 == lo0 or cs >= 512)
                            nc.tensor.matmul(
                                opt[:, cs % 512 : cs % 512 + (ce - cs)],
                                lhsT=v3[:, j0 : j0 + 2, 0 : DK + 1],
                                rhs=p3[:, :, cs:ce],
                                start=first,
                                stop=(m == last_m[g] and ce == min(w, 512 * (g + 1))),
                                perf_mode=mybir.MatmulPerfMode.DoubleRow,
                                skip_group_check=True,
                            )
                            cs = ce

                    if prev_pv is not None:
                        prev_pv()
                        if final and m - 1 == last_m[0]:
                            # bank 0 fully accumulated: drain + normalize it
                            # while the tail PVs of bank 1 still run
                            nc.vector.tensor_copy(ot[:, 0:512], opA[:, 0:512])
                            _epi_piece(0, stq_f)
                    prev_pv = _pv
                if prev_pv is not None:
                    prev_pv()
                if not final:
                    nc.vector.tensor_copy(ot[:, 0:512], opA[:, 0:512])
                nc.vector.tensor_copy(ot[:, 512:w], opB[:, 0 : w - 512])
                if final:
                    _epi_piece(1, stq_f)
                    nc.sync.dma_start(
                        out=stats_in[b, 1].rearrange("c p t -> p c t"),
                        in_=stq_f[:].rearrange("p (c t) -> p c t", t=NTH),
                    )
                    if fake_ar:
                        nc.sync.dma_start(out=stats_out[b, 1], in_=stats_in[b, 1])
                    else:
                        nc.gpsimd.collective_compute(
                            "AllReduce",
                            A.add,
                            replica_groups=rg,
                            ins=[stats_in[b, 1].opt()],
                            outs=[stats_out[b, 1].opt()],
                        )
                    emit_ln(b, 1, y_b)
                    return None

                def _epilogue():
                    nk = he // 128 - hs // 128
                    tps = [
                        ps.tile([128, 512], F32, tag="op", bufs=2, name=f"tp{g}")
                        for g in range((nk + 3) // 4)
                    ]
                    for i in range(hs // 128, he // 128):
                        k = i - hs // 128
                        tp = tps[k // 4]
                        nc.tensor.transpose(
                            tp[:, 128 * (k % 4) : 128 * (k % 4) + DK + 1],
                            ot[:, 128 * i - hs : 128 * i - hs + 128],
                            idn_t[:],
                        )
                    r8 = sb.tile([128, 8], F32, tag="r8", bufs=3)
                    for g, tp in enumerate(tps):
                        gn = min(4, nk - 4 * g)
                        nc.vector.reciprocal(
                            r8[:, 4 * g : 4 * g + gn],
                            tp[:].rearrange("q (k c) -> q k c", c=128)[
                                :, 0:gn, DK : DK + 1
                            ],
                        )
                    for i in range(hs // 128, he // 128):
                        k = i - hs // 128
                        tp = tps[k // 4]
                        nc.vector.scalar_tensor_tensor(
                            y_b[:, 128 * i + DK * hh : 128 * i + DK * hh + DK],
                            tp[:, 128 * (k % 4) : 128 * (k % 4) + DK],
                            r8[:, k : k + 1],
                            xs_b[:, 128 * i + DK * hh : 128 * i + DK * hh + DK],
                            A.mult,
                            A.add,
                            accum_out=acc_h[:, i : i + 1],
                        )
                    if hh == HPC - 1:
                        _emit_stats(b, y_b, accs, hs // 1024)
                        ln_ready.append((b, hs // 1024))

                return _epilogue

            emit_proj(0)
            pending = []
            ln_ready = []
            for pair in range(NP):
                for k, hs in enumerate(range(0, S_, 1024)):
                    hook = None
                    if k == 0 and pair + 1 < NP:
                        hook = (lambda pr=pair: emit_proj(pr + 1))
                    svc = None
                    if pair == NP - 1 and k == 1:
                        # pop the last pair's h0 epilogue mid-h1 so its LN
                        # stats chain overlaps the remaining exp tiles
                        svc = lambda: pending.pop(0)() if pending else None
                    epi = emit_jhalf(pair, hs, mid_hook=hook, service_hook=svc)
                    if epi is not None:
                        pending.append(epi)
                    if len(pending) > EPI_LAG:
                        pending.pop(0)()
                    if pair >= NP - 2:
                        # drain earlier batches' LN during the last two pairs
                        # so only the final batch's LN remains in the tail
                        for _ in range(2):
                            if ln_ready:
                                bb, cc = ln_ready.pop(0)
                                emit_ln(bb, cc, y_tiles[bb])
            for e in pending:
                e()
            while ln_ready:
                bb, cc = ln_ready.pop(0)
                emit_ln(bb, cc, y_tiles[bb], split=not ln_ready)


    # Restrict Exp/Ln to the shared natural_log_exp_and_others table set so
    # the whole kernel uses one ACT table load (indices preserved).
    import concourse.bacc as _bacc_mod

    _orig_tables = _bacc_mod.get_activation_tables

    def _filtered_tables(arch):
        out = {}
        for name, fns in _orig_tables(arch).items():
            if name != "natural_log_exp_and_others":
                fns = set(fns) - {AF.Exp, AF.Ln}
            out[name] = fns
        return out

    _bacc_mod.get_activation_tables = _filtered_tables
    try:
        nc.compile()
    finally:
        _bacc_mod.get_activation_tables = _orig_tables
    return nc


_GRAPH_CACHE = {}


def _get_graph(apply_affine: bool) -> bass.Bass:
    if apply_affine not in _GRAPH_CACHE:
        _GRAPH_CACHE[apply_affine] = _build_graph(apply_affine)
    return _GRAPH_CACHE[apply_affine]


def _prep_in_maps(x, Wq, bq, Wk, bk, Wv, bv, gamma, beta, apply_affine):
    scale = 1.0 / np.sqrt(np.float32(DK))
    in_maps = []
    for i in range(NCORES):
        dsl = slice(DC * i, DC * (i + 1))
        hsl = slice(HPC * i, HPC * (i + 1))
        x_sl = x[:, :, dsl]
        xt_full = x_sl.transpose(0, 2, 1).reshape(x.shape[0], HPC, DK, x.shape[1])
        xt_aug = np.concatenate(
            [xt_full, np.ones((x.shape[0], HPC, 1, x.shape[1]), np.float32)], axis=2
        )
        Wq_s = (Wq[hsl] * scale).astype(np.float64)
        bq_s = (bq[hsl] * scale).astype(np.float64)
        Wk_h = Wk[hsl].astype(np.float64)
        bk_h = bk[hsl].astype(np.float64)
        M = np.einsum("hde,hfe->hdf", Wk_h, Wq_s)      # [h, dK, dQ]
        u = np.einsum("hde,he->hd", Wk_h, bq_s)        # alpha coeffs (per t)
        wvec = np.einsum("hde,he->hd", Wq_s, bk_h)     # beta coeffs (per s)
        cconst = np.einsum("he,he->h", bk_h, bq_s)
        # lhsT for z: [d', dK | wvec]; z rows 0..63 = M@xh^T + u, row 64 = xh.w + c
        zw_np = np.concatenate(
            [M.transpose(0, 2, 1), wvec[:, :, None]], axis=2
        )  # [h, d'(=dQ... contraction dim), dK+1]
        zb_np = np.concatenate([u, cconst[:, None]], axis=1)[:, :, None]
        m = {
            "xt": np.ascontiguousarray(xt_aug).astype(BF_NP),
            "xs": np.ascontiguousarray(x_sl),
            "wpack": np.ascontiguousarray(
                np.concatenate(
                    [zw_np[0], zw_np[1], Wv[hsl][0], Wv[hsl][1]], axis=1
                )
            ).astype(BF_NP),
            "zb": np.ascontiguousarray(zb_np[:, :, 0].T).astype(np.float32),
            "bv16": np.ascontiguousarray(
                np.tile(bv[hsl][:, None, :], (1, 128, 16))
            ).astype(np.float32),
        }
        if apply_affine:
            m["gam"] = np.ascontiguousarray(
                np.tile(gamma[dsl][None, :], (128, 1))
            ).astype(np.float32)
            m["bet"] = np.ascontiguousarray(
                np.tile(beta[dsl][None, :], (128, 1))
            ).astype(np.float32)
        in_maps.append(m)
    return in_maps


def kernel(x, Wq, bq, Wk, bk, Wv, bv, gamma, beta):
    global LAST_RESULTS
    x = np.asarray(x, np.float32)
    Wq = np.asarray(Wq, np.float32)
    bq = np.asarray(bq, np.float32)
    Wk = np.asarray(Wk, np.float32)
    bk = np.asarray(bk, np.float32)
    Wv = np.asarray(Wv, np.float32)
    bv = np.asarray(bv, np.float32)
    gamma = np.asarray(gamma, np.float32)
    beta = np.asarray(beta, np.float32)

    apply_affine = not (
        np.allclose(gamma, 1.0, atol=0.0, rtol=0.0)
        and np.allclose(beta, 0.0, atol=0.0, rtol=0.0)
    )
    fake_ar = bool(int(os.environ.get("KERNEL_FAKE_AR", "0")))
    nc = _get_graph(apply_affine) if not fake_ar else _build_graph(apply_affine, fake_ar=True)

    in_maps = _prep_in_maps(x, Wq, bq, Wk, bk, Wv, bv, gamma, beta, apply_affine)

    res = run_bass_kernel_spmd(
        nc,
        in_maps,
        core_ids=list(range(NCORES)),
        trace=bool(int(os.environ.get("KERNEL_TRACE", "0"))),
    )
    LAST_RESULTS = res
    outs = [np.asarray(r["out"], np.float32) for r in res.results]
    return np.concatenate(outs, axis=2)


if __name__ == "__main__":
    nc = _build_graph(False)
    print("graph built ok:", len(nc.inst_map), "instructions")

